# revision 1
# baseline (speedup 1.0000x reference)
"""Trainium2 Bass kernel for Angles2Backbone (NeRF chain forward).

Full inputs: input [256,3,512] f32, param [6] f32, angles_length [256] i32.
Output: [256, 4608] f32  (coords of 1536 backbone atoms x 3, masked).

Sharding: pure data parallel over batch - 32 proteins per core x 8 cores.

Per-core algorithm (v2, residue-granularity scan):
  - Layout: 128 partitions = (quarter q in 0..3)*32 + protein b. Each row
    owns 128 consecutive residues (=384 atoms) of protein b's chain.
  - Pre-pass: per-residue product Rres = B_N @ B_CA @ B_C computed from
    cos/sin planes with per-type param scalars folded in (leaf structure
    collapses most terms into tensor_scalar/scalar_tensor_tensor ops).
  - Rotation prefix over 128 residues via Hillis-Steele (7 steps) on 9
    entry planes, ping-pong buffered, DVE/Pool split.
  - Cross-quarter fixup: gather quarter-end matrices, 3-step mini-scan,
    apply incoming prefix as per-partition scalars.
  - Atom translations: u_a = R_a * (first column of atom-level prefix)
    expands from residue prefixes with precomputed v-vectors; per-row
    prefix sum via hardware tensor_tensor_scan; additive cross-quarter
    fixup + length mask fused into the final store.
"""

import sys

sys.path.insert(0, "/opt/trn_rl_repo")

import numpy as np
import concourse.bass as bass
import concourse.bacc as bacc
import concourse.mybir as mybir
from concourse import tile
from concourse.bass_utils import run_bass_kernel_spmd

F32 = mybir.dt.float32
I32 = mybir.dt.int32
AF = mybir.ActivationFunctionType
OP = mybir.AluOpType

NCORES = 8
BPC = 32          # proteins per core
L = 512           # residues per protein
QN = 4            # chain quarters per protein (partition groups)
W = 384           # atoms per quarter
NR = 128          # residues per quarter (scan length)
PI = float(np.pi)

_CACHE = {}


def _e(i, k):
    return 3 * i + k


def _build_graph():
    nc = bacc.Bacc("TRN2", target_bir_lowering=False, debug=False,
                   num_devices=NCORES)
    inp = nc.dram_tensor("input", [QN * BPC, 3 * NR], F32,
                     kind="ExternalInput").ap()
    par = nc.dram_tensor("param", [6], F32, kind="ExternalInput").ap()
    alen = nc.dram_tensor("angles_length", [BPC], I32,
                          kind="ExternalInput").ap()
    out = nc.dram_tensor("out", [QN * BPC, 3 * W], F32,
                     kind="ExternalOutput").ap()

    with tile.TileContext(nc) as tc:
        _emit(nc, tc, inp, par, alen, out)
    nc.compile()
    return nc


def _emit(nc, tc, inp, par, alen, out):
    import contextlib
    ctx = contextlib.ExitStack()
    with ctx:
        main = ctx.enter_context(tc.tile_pool(name="main", bufs=1))
        tmps = ctx.enter_context(tc.tile_pool(name="tmps", bufs=24))
        psum = ctx.enter_context(tc.tile_pool(name="psum", bufs=1,
                                              space="PSUM"))

        # ---------------- persistent tiles ----------------
        alpha = main.tile([128, W], F32, tag="alpha")
        ca = main.tile([128, W], F32, tag="ca")
        sa = main.tile([128, W], F32, tag="sa")
        C1 = main.tile([128, 9 * NR], F32, tag="C1")
        RA = main.tile([128, 9 * NR], F32, tag="RA")   # residue mats / Pfull
        RB = main.tile([128, 9 * NR], F32, tag="RB")   # ping-pong
        PP = main.tile([128, 4 * NR], F32, tag="PP")   # pp1..pp4
        QQ = main.tile([128, 6 * NR], F32, tag="QQ")   # q1_i, q2_i
        Vm = main.tile([128, 9 * NR], F32, tag="Vm")   # v1,v2,v3 x 3 coords
        zeros = main.tile([128, W], F32, tag="zeros")
        ones = main.tile([128, NR], F32, tag="ones")
        Pall = main.tile([128, 3 * W], F32, tag="Pall")
        Pmall = main.tile([128, 3 * W], F32, tag="Pmall")
        jplane_i = main.tile([128, W], I32, tag="jplane_i")
        jplane = main.tile([128, W], F32, tag="jplane")
        maskp = main.tile([128, W], F32, tag="maskp")
        thr = main.tile([128, 1], F32, tag="thr")
        Lbc = main.tile([128, 1], F32, tag="Lbc")
        Lsb = main.tile([BPC, 1], I32, tag="Lsb")
        Lf = main.tile([BPC, 1], F32, tag="Lf")
        Psb = main.tile([1, 6], F32, tag="Psb")
        kv = main.tile([1, 3], F32, tag="kv")
        Rv = main.tile([1, 3], F32, tag="Rv")
        NSC = 24
        vecs = main.tile([1, NSC], F32, tag="vecs")
        Vb = main.tile([128, NSC], F32, tag="Vb")
        Estack = main.tile([BPC, 36], F32, tag="Estack")
        Fstack = main.tile([BPC, 27], F32, tag="Fstack")
        Fbc = main.tile([128, 9], F32, tag="Fbc")
        pestage = main.tile([BPC, 9], F32, tag="pestage")
        cumst = main.tile([BPC, 9], F32, tag="cumst")
        Pincb = main.tile([128, 3], F32, tag="Pincb")
        zb1 = main.tile([1, 1], F32, tag="zb1")
        zb128 = main.tile([128, 1], F32, tag="zb128")

        _cnt = [0]

        def ENG():
            # TT ops only: alternate DVE (2/3) and Pool (1/3)
            _cnt[0] += 1
            return nc.gpsimd if (_cnt[0] % 3 == 0) else nc.vector

        # ---------------- input DMA: assemble alpha ----------------
        # inp viewed as [c][q][b][m] so one DMA covers all 4 quarters
        # (dst partition order is (q, b), matching the AP walk order).
        av = alpha[:]
        nc.scalar.dma_start(Psb[:], par[:])
        nc.scalar.dma_start(Lsb[:], alen[:])
        nc.sync.dma_start(av[:, :], inp[:])

        # ---------------- param scalars ----------------
        for t, idx in enumerate((5, 1, 3)):   # kappa: CA_C_N, C_N_CA, N_CA_C
            nc.vector.tensor_copy(kv[0:1, t:t + 1], Psb[0:1, idx:idx + 1])
        for t, idx in enumerate((4, 0, 2)):   # R: R_C_N, R_N_CA, R_CA_C
            nc.vector.tensor_copy(Rv[0:1, t:t + 1], Psb[0:1, idx:idx + 1])
        nc.vector.memset(zb1[:], 0.0)
        nc.vector.memset(zb128[:], 0.0)
        # per-type ck/sk: sk=sin(kappa) (kappa in (0,pi)); ck=1-2sin^2(k/2)
        sk3 = main.tile([1, 3], F32, tag="sk3")
        ck3 = main.tile([1, 3], F32, tag="ck3")
        kvr = main.tile([1, 3], F32, tag="kvr")
        nc.scalar.activation(sk3[:], kv[0:1, 0:3], AF.Sin, bias=zb1[:])
        nc.scalar.activation(kvr[:], kv[0:1, 0:3], AF.Sin, bias=zb1[:],
                             scale=0.5)
        nc.scalar.square(kvr[:], kvr[:])
        nc.vector.tensor_scalar(ck3[:], kvr[:], -2.0, 1.0,
                                op0=OP.mult, op1=OP.add)

        # scalar slot layout in vecs[1, NSC]:
        # 0:ckN 1:skN 2:ckA 3:skA 4:ckC 5:skC
        # 6:ckNckA 7:skNskA 8:ckNskA 9:skNckA
        # 10:nskNckA 11:nckNskA 12:nckN 13:nckA 14:nckC 15:nskA
        # 16:RNckN 17:RNskN 18:RCA 19:RC
        def vc(i):
            return vecs[0:1, i:i + 1]

        for t in range(3):
            nc.vector.tensor_copy(vc(2 * t), ck3[0:1, t:t + 1])
            nc.vector.tensor_copy(vc(2 * t + 1), sk3[0:1, t:t + 1])
        nc.vector.tensor_mul(vc(6), vc(0), vc(2))     # ckN*ckA
        nc.vector.tensor_mul(vc(7), vc(1), vc(3))     # skN*skA
        nc.vector.tensor_mul(vc(8), vc(0), vc(3))     # ckN*skA
        nc.vector.tensor_mul(vc(9), vc(1), vc(2))     # skN*ckA
        nc.vector.tensor_scalar_mul(vc(10), vc(9), -1.0)
        nc.vector.tensor_scalar_mul(vc(11), vc(8), -1.0)
        nc.vector.tensor_scalar_mul(vc(12), vc(0), -1.0)
        nc.vector.tensor_scalar_mul(vc(13), vc(2), -1.0)
        nc.vector.tensor_scalar_mul(vc(14), vc(4), -1.0)
        nc.vector.tensor_scalar_mul(vc(15), vc(3), -1.0)
        nc.vector.tensor_mul(vc(16), Rv[0:1, 0:1], vc(0))   # RN*ckN
        nc.vector.tensor_mul(vc(17), Rv[0:1, 0:1], vc(1))   # RN*skN
        nc.vector.tensor_copy(vc(18), Rv[0:1, 1:2])         # R_CA
        nc.vector.tensor_copy(vc(19), Rv[0:1, 2:3])         # R_C
        nc.gpsimd.partition_broadcast(Vb[:], vecs[:])

        # selector matrices for PE-based cross-partition gather/broadcast
        rowid_i = main.tile([128, 1], I32, tag="rowid_i")
        rowid = main.tile([128, 1], F32, tag="rowid")
        colid = main.tile([128, 32], I32, tag="colid")
        rowq = main.tile([128, 1], F32, tag="rowq")
        I32f = main.tile([BPC, BPC], F32, tag="I32f")
        selq = main.tile([128, 4 * BPC], F32, tag="selq")
        nc.gpsimd.iota(rowid_i[:], [[0, 1]], channel_multiplier=1)
        nc.gpsimd.iota(colid[:], [[1, BPC]], channel_multiplier=0)
        nc.vector.tensor_copy(rowid[:], rowid_i[:])
        nc.vector.tensor_scalar(I32f[0:BPC, 0:BPC], colid[0:BPC, :],
                                rowid[0:BPC, 0:1], None, op0=OP.is_equal)
        for q in range(QN):
            nc.vector.tensor_scalar(rowq[:], rowid[:], float(q * BPC), None,
                                    op0=OP.subtract)
            nc.vector.tensor_scalar(selq[:, q * BPC:(q + 1) * BPC], colid[:],
                                    rowq[:, 0:1], None, op0=OP.is_equal)
        PSg = psum.tile([BPC, 36], F32, tag="PSg")
        PSf = psum.tile([128, 9], F32, tag="PSf")
        PSp = psum.tile([BPC, 9], F32, tag="PSp")
        PSi = psum.tile([128, 3], F32, tag="PSi")

        S = {}
        for i, nm in enumerate(("ckN", "skN", "ckA", "skA", "ckC", "skC",
                                "ckNckA", "skNskA", "ckNskA", "skNckA",
                                "nskNckA", "nckNskA", "nckN", "nckA",
                                "nckC", "nskA", "RNckN", "RNskN",
                                "RCA", "RC")):
            S[nm] = Vb[:, i:i + 1]

        # trig: |alpha| < 4pi: s4=sin(a/4), c4=1-2sin^2(a/8);
        # s2=2*s4*c4, c2=1-2*s4^2; s1=2*s2*c2, c1=1-2*s2^2.
        # One chain per 128-col type block, pipelined across ACT/DVE.
        for t in range(3):
            bs = slice(t * NR, (t + 1) * NR)
            avb, cab, sab = av[:, bs], ca[:, bs], sa[:, bs]
            ts8 = tmps.tile([128, NR], F32, tag="t1")
            ts4 = tmps.tile([128, NR], F32, tag="t2")
            tq = tmps.tile([128, NR], F32, tag="t1")
            nc.scalar.activation(ts8[:], avb, AF.Sin, bias=zb128[:],
                                 scale=0.125)
            nc.scalar.activation(ts4[:], avb, AF.Sin, bias=zb128[:],
                                 scale=0.25)
            nc.scalar.square(ts8[:], ts8[:])
            nc.vector.tensor_scalar(cab, ts8[:], -2.0, 1.0,
                                    op0=OP.mult, op1=OP.add)          # c4
            nc.vector.scalar_tensor_tensor(ts8[:], ts4[:], 2.0, cab,
                                           op0=OP.mult, op1=OP.mult)  # s2
            nc.scalar.square(tq[:], ts4[:])
            nc.vector.tensor_scalar(ts4[:], tq[:], -2.0, 1.0,
                                    op0=OP.mult, op1=OP.add)          # c2
            nc.vector.scalar_tensor_tensor(sab, ts8[:], 2.0, ts4[:],
                                           op0=OP.mult, op1=OP.mult)  # s1
            nc.scalar.square(tq[:], ts8[:])
            nc.vector.tensor_scalar(cab, tq[:], -2.0, 1.0,
                                    op0=OP.mult, op1=OP.add)          # c1
        nc.gpsimd.memset(zeros[:], 0.0)
        nc.gpsimd.memset(zeros[:], 0.0)
        nc.gpsimd.memset(ones[:], 1.0)

        # per-type strided cos/sin views [128, 128]
        cN, sN = ca[:, 0:128], sa[:, 0:128]
        cA, sA = ca[:, 128:256], sa[:, 128:256]
        cC, sC = ca[:, 256:384], sa[:, 256:384]

        def blk(t, e, lo=0, hi=NR):
            return t[:, e * NR + lo:e * NR + hi]

        V = nc.vector
        STT = nc.vector.scalar_tensor_tensor
        TS = nc.vector.tensor_scalar

        # ---------------- pre-pass: C1 = B_N @ B_CA ----------------
        pp1 = PP[:, 0 * NR:1 * NR]
        pp2 = PP[:, 1 * NR:2 * NR]
        pp3 = PP[:, 2 * NR:3 * NR]
        pp4 = PP[:, 3 * NR:4 * NR]
        nc.gpsimd.tensor_mul(pp1, cN, cA)
        nc.gpsimd.tensor_mul(pp2, sN, sA)
        nc.gpsimd.tensor_mul(pp3, cN, sA)
        nc.gpsimd.tensor_mul(pp4, sN, cA)
        c1 = C1[:]
        TS(blk(c1, 0), cA, S["skNskA"], S["ckNckA"],
           op0=OP.mult, op1=OP.add)                       # C1_00
        TS(blk(c1, 1), cA, S["nskNckA"], S["ckNskA"],
           op0=OP.mult, op1=OP.add)                       # C1_01
        V.tensor_scalar_mul(blk(c1, 2), sA, S["skN"])     # C1_02
        x = blk(c1, 3)                                    # C1_10
        nc.scalar.mul(x, cN, S["skNckA"])
        STT(x, pp1, S["nckNskA"], x, op0=OP.mult, op1=OP.add)
        STT(x, pp2, S["skA"], x, op0=OP.mult, op1=OP.add)
        x = blk(c1, 4)                                    # C1_11
        nc.scalar.mul(x, cN, S["skNskA"])
        STT(x, pp1, S["ckNckA"], x, op0=OP.mult, op1=OP.add)
        STT(x, pp2, S["nckA"], x, op0=OP.mult, op1=OP.add)
        STT(blk(c1, 5), pp3, S["nckN"], pp4,
            op0=OP.mult, op1=OP.subtract)                 # C1_12
        x = blk(c1, 6)                                    # C1_20
        nc.scalar.mul(x, sN, S["skNckA"])
        STT(x, pp4, S["nckNskA"], x, op0=OP.mult, op1=OP.add)
        STT(x, pp3, S["nskA"], x, op0=OP.mult, op1=OP.add)
        x = blk(c1, 7)                                    # C1_21
        nc.scalar.mul(x, sN, S["skNskA"])
        STT(x, pp4, S["ckNckA"], x, op0=OP.mult, op1=OP.add)
        STT(x, pp3, S["ckA"], x, op0=OP.mult, op1=OP.add)
        STT(blk(c1, 8), pp2, S["nckN"], pp1,
            op0=OP.mult, op1=OP.add)                      # C1_22

        # residue-0 of q=0: B_N := Identity => C1 := B_CA(0)
        # (alpha_CA(0)=0 so cA=1, sA=0 there): [[ckA,skA,0],[skA,-ckA,0],
        # [0,0,-1]]
        r0s = slice(0, BPC)
        o1 = ones[r0s, 0:1]
        V.tensor_scalar_mul(c1[r0s, 0 * NR:0 * NR + 1], o1, S["ckA"][r0s])
        V.tensor_scalar_mul(c1[r0s, 1 * NR:1 * NR + 1], o1, S["skA"][r0s])
        V.memset(c1[r0s, 2 * NR:2 * NR + 1], 0.0)
        V.tensor_scalar_mul(c1[r0s, 3 * NR:3 * NR + 1], o1, S["skA"][r0s])
        V.tensor_scalar_mul(c1[r0s, 4 * NR:4 * NR + 1], o1, S["nckA"][r0s])
        V.memset(c1[r0s, 5 * NR:5 * NR + 1], 0.0)
        V.memset(c1[r0s, 6 * NR:6 * NR + 1], 0.0)
        V.memset(c1[r0s, 7 * NR:7 * NR + 1], 0.0)
        V.memset(c1[r0s, 8 * NR:8 * NR + 1], -1.0)

        # ---------------- pre-pass: Rres = C1 @ B_C -> RA ----------------
        ra = RA[:]
        for i in range(3):
            nc.gpsimd.tensor_mul(blk(QQ[:], i), blk(c1, _e(i, 1)), cC)
            nc.gpsimd.tensor_mul(blk(QQ[:], 3 + i), blk(c1, _e(i, 2)), sC)
        for i in range(3):
            q1i = blk(QQ[:], i)
            q2i = blk(QQ[:], 3 + i)
            x = blk(ra, _e(i, 0))
            nc.scalar.mul(x, blk(c1, _e(i, 0)), S["ckC"])
            STT(x, q1i, S["skC"], x, op0=OP.mult, op1=OP.add)
            STT(x, q2i, S["skC"], x, op0=OP.mult, op1=OP.add)
            x = blk(ra, _e(i, 1))
            nc.scalar.mul(x, blk(c1, _e(i, 0)), S["skC"])
            STT(x, q1i, S["nckC"], x, op0=OP.mult, op1=OP.add)
            STT(x, q2i, S["nckC"], x, op0=OP.mult, op1=OP.add)
            t1 = tmps.tile([128, NR], F32, tag="pt1")
            t2 = tmps.tile([128, NR], F32, tag="pt2")
            nc.gpsimd.tensor_mul(t1[:], blk(c1, _e(i, 1)), sC)
            nc.gpsimd.tensor_mul(t2[:], blk(c1, _e(i, 2)), cC)
            nc.gpsimd.tensor_sub(blk(ra, _e(i, 2)), t1[:], t2[:])

        # ---------------- v-vectors for atom expansion ----------------
        # v1 = t_N = RN*(ckN, skN*cN, skN*sN); v2 = RCA*C1[:,0];
        # v3 = RC*Rres[:,0]
        vm = Vm[:]
        nc.scalar.mul(blk(vm, 0), ones[:], S["RNckN"])
        nc.scalar.mul(blk(vm, 1), cN, S["RNskN"])
        nc.scalar.mul(blk(vm, 2), sN, S["RNskN"])
        for i in range(3):
            nc.scalar.mul(blk(vm, 3 + i), blk(c1, _e(i, 0)), S["RCA"])
            nc.scalar.mul(blk(vm, 6 + i), blk(ra, _e(i, 0)), S["RC"])

        # ---------------- Hillis-Steele residue scan ----------------
        # Fused step: all 9 output entries in one 3-dim AP op per k-term:
        #   out[i,j] += L[i,k] (bcast over j) * R[k,j] (bcast over i)
        # 5 logical ops per step, each split col-wise DVE/Pool.
        def ap3(base_ap, off, dims):
            return bass.AP(base_ap.tensor, base_ap.offset + off,
                           [list(base_ap.ap[0])] + [list(d) for d in dims])

        T9a = main.tile([128, 9 * NR], F32, tag="T9a")
        T9b = main.tile([128, 9 * NR], F32, tag="T9b")

        def fused_step(srcb, dstb, s, nr):
            n = nr - s
            cut = (n * 3) // 4          # DVE share of columns
            sv = srcb.rearrange("p (e j) -> p e j", e=9)
            dv = dstb.rearrange("p (e j) -> p e j", e=9)
            nc.scalar.copy(dv[:, :, 0:s], sv[:, :, 0:s])

            def L(k, c0, c1):
                return ap3(srcb, k * nr + c0,
                           [[3 * nr, 3], [0, 3], [1, c1 - c0]])

            def R(k, c0, c1):
                return ap3(srcb, 3 * k * nr + s + c0,
                           [[0, 3], [nr, 3], [1, c1 - c0]])

            def T(t, c0, c1):
                return ap3(t[:], c0, [[3 * nr, 3], [nr, 3], [1, c1 - c0]])

            def O(c0, c1):
                return ap3(dstb, s + c0, [[3 * nr, 3], [nr, 3], [1, c1 - c0]])

            for E, c0, c1 in ((nc.vector, 0, cut), (nc.gpsimd, cut, n)):
                if c1 <= c0:
                    continue
                E.tensor_mul(T(T9a, c0, c1), L(0, c0, c1), R(0, c0, c1))
                E.tensor_mul(T(T9b, c0, c1), L(1, c0, c1), R(1, c0, c1))
                E.tensor_add(T(T9a, c0, c1), T(T9a, c0, c1), T(T9b, c0, c1))
                E.tensor_mul(T(T9b, c0, c1), L(2, c0, c1), R(2, c0, c1))
                E.tensor_add(O(c0, c1), T(T9a, c0, c1), T(T9b, c0, c1))

        # pair adjacent residues: P2[r'] = Rres[2r'] @ Rres[2r'+1]
        NR2 = NR // 2
        P2A = main.tile([128, 9 * NR2], F32, tag="P2A")
        P2B = main.tile([128, 9 * NR2], F32, tag="P2B")
        pcut = (NR2 * 3) // 4
        ra_ap = RA[:]

        def PL(k, c0, c1):
            return ap3(ra_ap, k * NR + 2 * c0,
                       [[3 * NR, 3], [0, 3], [2, c1 - c0]])

        def PR(k, c0, c1):
            return ap3(ra_ap, 3 * k * NR + 1 + 2 * c0,
                       [[0, 3], [NR, 3], [2, c1 - c0]])

        def PT(t, c0, c1):
            return ap3(t[:], c0, [[3 * NR2, 3], [NR2, 3], [1, c1 - c0]])

        def PO(c0, c1):
            return ap3(P2A[:], c0, [[3 * NR2, 3], [NR2, 3], [1, c1 - c0]])

        for E, c0, c1 in ((nc.vector, 0, pcut), (nc.gpsimd, pcut, NR2)):
            E.tensor_mul(PT(T9a, c0, c1), PL(0, c0, c1), PR(0, c0, c1))
            E.tensor_mul(PT(T9b, c0, c1), PL(1, c0, c1), PR(1, c0, c1))
            E.tensor_add(PT(T9a, c0, c1), PT(T9a, c0, c1), PT(T9b, c0, c1))
            E.tensor_mul(PT(T9b, c0, c1), PL(2, c0, c1), PR(2, c0, c1))
            E.tensor_add(PO(c0, c1), PT(T9a, c0, c1), PT(T9b, c0, c1))

        Wodd = main.tile([128, 9 * NR2], F32, tag="Wodd")
        T9c = main.tile([128, 9 * NR2], F32, tag="T9c")
        T9d = main.tile([128, 9 * NR2], F32, tag="T9d")
        wo = Wodd[:]
        wcut = (NR2 * 3) // 4

        def WL(k, c0, c1):
            return ap3(ra_ap, k * NR + 2 * c0,
                       [[3 * NR, 3], [0, 3], [2, c1 - c0]])

        def WR(k, c0, c1):
            return ap3(vm, k * NR + 1 + 2 * c0,
                       [[0, 3], [3 * NR, 3], [2, c1 - c0]])

        def WT(t, c0, c1):
            return ap3(t[:], c0, [[3 * NR2, 3], [NR2, 3], [1, c1 - c0]])

        def WO(c0, c1):
            return ap3(wo, c0, [[NR2, 3], [3 * NR2, 3], [1, c1 - c0]])

        for E, c0, c1 in ((nc.vector, 0, wcut), (nc.gpsimd, wcut, NR2)):
            E.tensor_mul(WT(T9a, c0, c1), WL(0, c0, c1), WR(0, c0, c1))
            E.tensor_mul(WT(T9b, c0, c1), WL(1, c0, c1), WR(1, c0, c1))
            E.tensor_add(WT(T9a, c0, c1), WT(T9a, c0, c1), WT(T9b, c0, c1))
            E.tensor_mul(WT(T9b, c0, c1), WL(2, c0, c1), WR(2, c0, c1))
            E.tensor_add(WO(c0, c1), WT(T9a, c0, c1), WT(T9b, c0, c1))


        # quad level: P4[r''] = P2[2r''] @ P2[2r''+1]
        NR4 = NR2 // 2
        P4A = main.tile([128, 9 * NR4], F32, tag="P4A")
        P4B = main.tile([128, 9 * NR4], F32, tag="P4B")
        W2 = main.tile([128, 18 * NR4], F32, tag="W2")
        p2a = P2A[:]
        w2 = W2[:]
        qcut = (NR4 * 3) // 4

        def QL(k, c0, c1):
            return ap3(p2a, k * NR2 + 2 * c0,
                       [[3 * NR2, 3], [0, 3], [2, c1 - c0]])

        def QR(k, c0, c1):
            return ap3(p2a, 3 * k * NR2 + 1 + 2 * c0,
                       [[0, 3], [NR2, 3], [2, c1 - c0]])

        def QT(t, c0, c1):
            return ap3(t[:], c0, [[3 * NR4, 3], [NR4, 3], [1, c1 - c0]])

        def QO(c0, c1):
            return ap3(P4A[:], c0, [[3 * NR4, 3], [NR4, 3], [1, c1 - c0]])

        for E, c0, c1 in ((nc.vector, 0, qcut), (nc.gpsimd, qcut, NR4)):
            E.tensor_mul(QT(T9a, c0, c1), QL(0, c0, c1), QR(0, c0, c1))
            E.tensor_mul(QT(T9b, c0, c1), QL(1, c0, c1), QR(1, c0, c1))
            E.tensor_add(QT(T9a, c0, c1), QT(T9a, c0, c1), QT(T9b, c0, c1))
            E.tensor_mul(QT(T9b, c0, c1), QL(2, c0, c1), QR(2, c0, c1))
            E.tensor_add(QO(c0, c1), QT(T9a, c0, c1), QT(T9b, c0, c1))

        # W2 group A (m~=0..2): P2_even @ (vm at residues 4r''+2)
        # W2 group B (m~=3..5): P2_even @ (Wodd at odd superblocks)
        def W2L(k, c0, c1):
            return ap3(p2a, k * NR2 + 2 * c0,
                       [[3 * NR2, 3], [0, 3], [2, c1 - c0]])

        def W2RA(k, c0, c1):
            return ap3(vm, k * NR + 2 + 4 * c0,
                       [[0, 3], [3 * NR, 3], [4, c1 - c0]])

        def W2RB(k, c0, c1):
            return ap3(wo, k * NR2 + 1 + 2 * c0,
                       [[0, 3], [3 * NR2, 3], [2, c1 - c0]])

        def W2O(goff, c0, c1):
            return ap3(w2, goff + c0, [[NR4, 3], [3 * NR4, 3], [1, c1 - c0]])

        for goff, RF in ((0, W2RA), (9 * NR4, W2RB)):
            for E, c0, c1 in ((nc.vector, 0, qcut), (nc.gpsimd, qcut, NR4)):
                E.tensor_mul(QT(T9c, c0, c1), W2L(0, c0, c1), RF(0, c0, c1))
                E.tensor_mul(QT(T9d, c0, c1), W2L(1, c0, c1), RF(1, c0, c1))
                E.tensor_add(QT(T9c, c0, c1), QT(T9c, c0, c1),
                             QT(T9d, c0, c1))
                E.tensor_mul(QT(T9d, c0, c1), W2L(2, c0, c1), RF(2, c0, c1))
                E.tensor_add(W2O(goff, c0, c1), QT(T9c, c0, c1),
                             QT(T9d, c0, c1))

        bufs = [P4A, P4B]
        nsteps = 5
        for step in range(nsteps):
            fused_step(bufs[step % 2][:], bufs[(step + 1) % 2][:],
                       1 << step, NR4)
        Rscan = bufs[nsteps % 2][:]    # RB: local residue prefixes

        # ---------------- cross-quarter rotation fixup ----------------
        for q in range(QN):
            nc.tensor.matmul(
                PSg[0:BPC, q * 9:(q + 1) * 9],
                selq[:, q * BPC:(q + 1) * BPC],
                Rscan[:, NR4 - 1:9 * NR4:NR4], start=True, stop=True)
        nc.vector.tensor_copy(Estack[0:BPC, 0:36], PSg[0:BPC, 0:36])
        nc.vector.tensor_copy(Fstack[0:BPC, 0:9], Estack[0:BPC, 0:9])
        mt0 = main.tile([BPC, 9], F32, tag="mt0")
        mt1 = main.tile([BPC, 9], F32, tag="mt1")
        fs = Fstack[:]
        es = Estack[:]

        def ap2(base_ap, off, dims):
            return bass.AP(base_ap.tensor, base_ap.offset + off,
                           [list(base_ap.ap[0])] + [list(d) for d in dims])

        for q in (1, 2):
            FL = lambda k: ap2(fs, (q - 1) * 9 + k, [[3, 3], [0, 3]])
            ER = lambda k: ap2(es, q * 9 + 3 * k, [[0, 3], [1, 3]])
            MT = lambda t: ap2(t[:], 0, [[3, 3], [1, 3]])
            FO = ap2(fs, q * 9, [[3, 3], [1, 3]])
            V.tensor_mul(MT(mt0), FL(0), ER(0))
            V.tensor_mul(MT(mt1), FL(1), ER(1))
            V.tensor_add(MT(mt0), MT(mt0), MT(mt1))
            V.tensor_mul(MT(mt1), FL(2), ER(2))
            V.tensor_add(FO, MT(mt0), MT(mt1))
        nc.vector.memset(Fbc[0:BPC, 0:9], 0.0)
        for e in (0, 4, 8):
            nc.vector.memset(Fbc[0:BPC, e:e + 1], 1.0)
        for q in (1, 2):
            nc.tensor.matmul(
                PSf[q * BPC:(q + 1) * BPC, 0:9], I32f[0:BPC, 0:BPC],
                Fstack[0:BPC, (q - 1) * 9:q * 9], start=True, stop=True)
        for q in (1, 2):
            nc.vector.tensor_copy(Fbc[q * BPC:(q + 1) * BPC, 0:9],
                                  PSf[q * BPC:(q + 1) * BPC, 0:9])
        nc.sync.dma_start(Fbc[3 * BPC:128, 0:9], Fstack[0:BPC, 18:27])
        # ---------------- atom translations (local frame) ----------------
        # superblock = 2 residues = 6 atoms. w_m (m=0..5): prefix-within-
        # superblock applied to t-vectors; m<3 are the per-residue v's at
        # even residues, m>=3 need Rres_even @ v_odd (fused below).
        Uloc = main.tile([128, 3 * W], F32, tag="Uloc")
        ul = Uloc[:]
        rs = Rscan
        # superblock 0: local prefix = identity -> u = w_m
        V.tensor_copy(ap3(ul, 0, [[W, 3], [1, 3]]),
                      ap3(vm, 0, [[NR, 3], [3 * NR, 3]]))
        V.tensor_copy(ap3(ul, 3, [[W, 3], [1, 3]]),
                      ap3(wo, 0, [[NR2, 3], [3 * NR2, 3]]))
        # superblock-0 atoms 6..11 from W2 (local prefix = identity)
        V.tensor_copy(ap3(ul, 6, [[W, 3], [1, 3]]),
                      ap3(w2, 0, [[NR4, 3], [3 * NR4, 3]]))
        V.tensor_copy(ap3(ul, 9, [[W, 3], [1, 3]]),
                      ap3(w2, 9 * NR4, [[NR4, 3], [3 * NR4, 3]]))
        nu = NR4 - 1
        ucut = (nu * 3) // 4

        def UL(k, c0, c1):
            return ap3(rs, k * NR4 + c0, [[3 * NR4, 3], [0, 3], [1, c1 - c0]])

        def UR0(k, c0, c1):   # atoms 12r''+0..2: vm at residue 4r''
            return ap3(vm, k * NR + 4 + 4 * c0,
                       [[0, 3], [3 * NR, 3], [4, c1 - c0]])

        def UR1(k, c0, c1):   # atoms +3..5: Wodd at superblock 2r''
            return ap3(wo, k * NR2 + 2 + 2 * c0,
                       [[0, 3], [3 * NR2, 3], [2, c1 - c0]])

        def UR2(k, c0, c1):   # atoms +6..8: W2 group A
            return ap3(w2, k * NR4 + 1 + c0,
                       [[0, 3], [3 * NR4, 3], [1, c1 - c0]])

        def UR3(k, c0, c1):   # atoms +9..11: W2 group B
            return ap3(w2, 9 * NR4 + k * NR4 + 1 + c0,
                       [[0, 3], [3 * NR4, 3], [1, c1 - c0]])

        def UT(t, c0, c1):
            return ap3(t[:], c0, [[3 * NR4, 3], [NR4, 3], [1, c1 - c0]])

        def UO(off, c0, c1):
            return ap3(ul, off + 12 * c0, [[W, 3], [1, 3], [12, c1 - c0]])

        for gi, (off, RF) in enumerate(((12, UR0), (15, UR1),
                                        (18, UR2), (21, UR3))):
            ta = [T9a, T9c][gi % 2]
            tb = [T9b, T9d][gi % 2]
            for E, c0, c1 in ((nc.vector, 0, ucut), (nc.gpsimd, ucut, nu)):
                E.tensor_mul(UT(ta, c0, c1), UL(0, c0, c1), RF(0, c0, c1))
                E.tensor_mul(UT(tb, c0, c1), UL(1, c0, c1), RF(1, c0, c1))
                E.tensor_add(UT(ta, c0, c1), UT(ta, c0, c1), UT(tb, c0, c1))
                E.tensor_mul(UT(tb, c0, c1), UL(2, c0, c1), RF(2, c0, c1))
                E.tensor_add(UO(off, c0, c1), UT(ta, c0, c1), UT(tb, c0, c1))
        # prefix-sum the LOCAL u per coordinate (frame fix applied at the
        # end by linearity: sum_j F@u = F@sum_j u)
        for c in range(3):
            uc = ul[:, c * W:(c + 1) * W]
            V.memset(uc[0:BPC, 0:1], 0.0)   # atom 0 of the whole chain
            nc.vector.tensor_tensor_scan(
                Pall[:, c * W:(c + 1) * W], uc, zeros[:], 0.0,
                op0=OP.add, op1=OP.add)

        # ---------------- cross-quarter translation fixup ----------------
        pv = Pall[:]
        for q in range(3):
            nc.tensor.matmul(
                PSp[0:BPC, q * 3:(q + 1) * 3],
                selq[:, q * BPC:(q + 1) * BPC],
                pv[:, W - 1:3 * W:W], start=True, stop=True)
        nc.vector.tensor_copy(pestage[0:BPC, 0:9], PSp[0:BPC, 0:9])
        # global pe_q = F_q @ pe_local_q (F_0 = I); Fstack block q-1 = F_q
        peg = main.tile([BPC, 9], F32, tag="peg")
        ps = pestage[:]
        nc.vector.tensor_copy(peg[0:BPC, 0:3], pestage[0:BPC, 0:3])
        for q in (1, 2):
            FL = lambda k: ap2(fs, (q - 1) * 9 + k, [[3, 3]])
            PR = lambda k: ap2(ps, q * 3 + k, [[0, 3]])
            M3 = lambda t: ap2(t[:], 0, [[1, 3]])
            PO = ap2(peg[:], q * 3, [[1, 3]])
            V.tensor_mul(M3(mt0), FL(0), PR(0))
            V.tensor_mul(M3(mt1), FL(1), PR(1))
            V.tensor_add(M3(mt0), M3(mt0), M3(mt1))
            V.tensor_mul(M3(mt1), FL(2), PR(2))
            V.tensor_add(PO, M3(mt0), M3(mt1))
        nc.vector.tensor_copy(cumst[0:BPC, 0:3], peg[0:BPC, 0:3])
        nc.vector.tensor_add(cumst[0:BPC, 3:6], cumst[0:BPC, 0:3],
                             peg[0:BPC, 3:6])
        nc.vector.tensor_add(cumst[0:BPC, 6:9], cumst[0:BPC, 3:6],
                             peg[0:BPC, 6:9])
        nc.vector.memset(Pincb[0:BPC, 0:3], 0.0)
        for q in (1, 2):
            nc.tensor.matmul(
                PSi[q * BPC:(q + 1) * BPC, 0:3], I32f[0:BPC, 0:BPC],
                cumst[0:BPC, (q - 1) * 3:q * 3], start=True, stop=True)
        for q in (1, 2):
            nc.vector.tensor_copy(Pincb[q * BPC:(q + 1) * BPC, 0:3],
                                  PSi[q * BPC:(q + 1) * BPC, 0:3])
        nc.scalar.dma_start(Pincb[3 * BPC:128, 0:3], cumst[0:BPC, 6:9])

        # ---------------- mask ----------------
        nc.gpsimd.iota(jplane_i[:], [[1, W]], channel_multiplier=0)
        nc.vector.tensor_copy(jplane[:], jplane_i[:])
        nc.vector.tensor_copy(Lf[:], Lsb[:])
        for q in range(QN):
            (nc.sync if q % 2 else nc.scalar).dma_start(
                Lbc[q * BPC:(q + 1) * BPC, 0:1], Lf[:])
        for q in range(QN):
            TS(thr[q * BPC:(q + 1) * BPC, 0:1],
               Lbc[q * BPC:(q + 1) * BPC, 0:1],
               3.0, float(q * W), op0=OP.mult, op1=OP.subtract)
        TS(maskp[:], jplane[:], thr[:, 0:1], None, op0=OP.is_lt)

        # ------------- fused frame-fix + P_inc + mask + store -------------
        for c in range(3):
            x = tmps.tile([128, W], F32, tag="t1")
            V.tensor_scalar_mul(x[:], pv[:, 0:W],
                                Fbc[:, _e(c, 0):_e(c, 0) + 1])
            STT(x[:], pv[:, W:2 * W], Fbc[:, _e(c, 1):_e(c, 1) + 1], x[:],
                op0=OP.mult, op1=OP.add)
            STT(x[:], pv[:, 2 * W:3 * W], Fbc[:, _e(c, 2):_e(c, 2) + 1], x[:],
                op0=OP.mult, op1=OP.add)
            STT(Pmall[:, c * W:(c + 1) * W], x[:],
                Pincb[:, c:c + 1], maskp[:], op0=OP.add, op1=OP.mult)
        for c in range(3):
            (nc.sync if c % 2 == 0 else nc.scalar).dma_start(
                out[:, c * W:(c + 1) * W], Pmall[:, c * W:(c + 1) * W])


def _prep_alpha(input):
    # pure indexing: alphaN[r]=psi[r-1], alphaCA[r]=omega[r-1] (0 at r=0),
    # alphaC[r]=phi[r]; blocked (q, b, type, m).
    phi, psi, om = input[:, 0], input[:, 1], input[:, 2]
    z1 = np.zeros((input.shape[0], 1), np.float32)
    aN = np.concatenate([z1, psi[:, :-1]], axis=1)
    aCA = np.concatenate([z1, om[:, :-1]], axis=1)
    alpha = np.stack([aN, aCA, phi], axis=1)          # [B, 3, 512]
    return alpha.reshape(-1, 3, QN, NR).transpose(0, 2, 1, 3)


def _shard_alpha(alpha, i):
    sl = slice(i * BPC, (i + 1) * BPC)
    return np.ascontiguousarray(
        alpha[sl].transpose(1, 0, 2, 3).reshape(QN * BPC, 3 * NR))


def _get_nc():
    if "nc" not in _CACHE:
        _CACHE["nc"] = _build_graph()
    return _CACHE["nc"]


def kernel(input, param, angles_length, trace=False):
    input = np.ascontiguousarray(input, dtype=np.float32)
    param = np.ascontiguousarray(param, dtype=np.float32)
    angles_length = np.ascontiguousarray(angles_length, dtype=np.int32)
    nc = _get_nc()
    alpha = _prep_alpha(input)
    in_maps = []
    for i in range(NCORES):
        sl = slice(i * BPC, (i + 1) * BPC)
        in_maps.append({
            "input": _shard_alpha(alpha, i),
            "param": param,
            "angles_length": angles_length[sl],
        })
    res = run_bass_kernel_spmd(nc, in_maps, core_ids=list(range(NCORES)),
                               trace=trace)
    outs = []
    for i in range(NCORES):
        r = res.results[i]["out"]          # [(q,b), (c,j)]
        r = r.reshape(QN, BPC, 3, W)
        r = np.transpose(r, (1, 0, 3, 2)).reshape(BPC, 3 * QN * W)
        outs.append(r)
    full = np.concatenate(outs, axis=0).astype(np.float32)
    if trace:
        kernel._last_exec_ns = res.exec_time_ns
    return full


kernel._last_exec_ns = None



# revision 22
# speedup vs baseline: 1.0673x; 1.0673x over previous
"""Trainium2 Bass kernel for Angles2Backbone (NeRF chain forward).

Full inputs: input [256,3,512] f32, param [6] f32, angles_length [256] i32.
Output: [256, 4608] f32  (coords of 1536 backbone atoms x 3, masked).

Sharding: pure data parallel over batch - 32 proteins per core x 8 cores.

Per-core algorithm (v3: fp16 scan machinery, mod-4 blocked residue layout):
  - 128 partitions = (quarter q)*32 + protein b; each row owns 128 residues.
  - Residue r of a quarter lives at column sigma(r) = 32*(r%4) + r//4
    (host-side permutation), so every pair/quad/expansion op reads and
    writes stride-1 column blocks -> DVE 2x fp16 mode throughout.
  - Trig via sin LUT at a/8, a/4 + double-angle chains (fp16).
  - Pre-pass builds per-residue rotation Rres (fp16) from scalar-folded
    bilinear terms; v-vectors (per-atom translations) in fp16.
  - pair (mod-4 blocks) -> P2, quad -> P4, Hillis-Steele over 32 quads.
  - Vector expansions Wodd/W2 written straight into a fused source tensor
    US so the superquad expansion is 5 big ops against a replicated,
    shifted prefix tensor L4.
  - Positions: blocked u -> reorder copy -> masked tensor_tensor_scan
    seeded with F^T.Pinc (cross-quarter fixup folded into the scan),
    then frame rotation F and store.
  - Cross-partition moves via PE matmuls only (no SBUF-SBUF DMA).
"""

import sys

sys.path.insert(0, "/opt/trn_rl_repo")

import numpy as np
import concourse.bass as bass
import concourse.bacc as bacc
import concourse.mybir as mybir
from concourse import tile
from concourse.bass_utils import run_bass_kernel_spmd

F32 = mybir.dt.float32
F16 = mybir.dt.float16
I32 = mybir.dt.int32
AF = mybir.ActivationFunctionType
OP = mybir.AluOpType

NCORES = 8
BPC = 32          # proteins per core
L = 512           # residues per protein
QN = 4            # chain quarters per protein (partition groups)
W = 384           # atoms per quarter
NR = 128          # residues per quarter
NB = 32           # columns per mod-4 block
PI = float(np.pi)

_CACHE = {}


def _build_graph():
    nc = bacc.Bacc("TRN2", target_bir_lowering=False, debug=False,
                   num_devices=NCORES)
    inp = nc.dram_tensor("input", [QN * BPC, 3 * NR], F32,
                         kind="ExternalInput").ap()
    par = nc.dram_tensor("param", [6], F32, kind="ExternalInput").ap()
    alen = nc.dram_tensor("angles_length", [BPC], I32,
                          kind="ExternalInput").ap()
    out = nc.dram_tensor("out", [QN * BPC, 3 * W], F32,
                         kind="ExternalOutput").ap()
    with tile.TileContext(nc) as tc:
        _emit(nc, tc, inp, par, alen, out)
    nc.compile()
    return nc


def _ap(base_ap, off, dims):
    return bass.AP(base_ap.tensor, base_ap.offset + off,
                   [list(base_ap.ap[0])] + [list(d) for d in dims])


def _emit(nc, tc, inp, par, alen, out):
    import contextlib
    ctx = contextlib.ExitStack()
    with ctx:
        main = ctx.enter_context(tc.tile_pool(name="main", bufs=1))
        psum = ctx.enter_context(tc.tile_pool(name="psum", bufs=1,
                                              space="PSUM"))

        # ---------------- tiles ----------------
        alpha = main.tile([128, 3 * NR], F32, tag="alpha")
        ca = main.tile([128, 3 * NR], F32, tag="ca")
        sa = main.tile([128, 3 * NR], F32, tag="sa")
        # trig scratch (per type block)
        ts8 = main.tile([128, 3 * NR], F32, tag="ts8")   # s8 then sq8
        ts4 = main.tile([128, 3 * NR], F32, tag="ts4")   # s4 then s4*2
        tsq = main.tile([128, 3 * NR], F32, tag="tsq")   # squares
        ts2 = main.tile([128, 3 * NR], F32, tag="ts2")   # s2 then s2*2

        PP = main.tile([128, 4 * NR], F32, tag="PP")
        C1 = main.tile([128, 9 * NR], F32, tag="C1")
        QQ = main.tile([128, 6 * NR], F32, tag="QQ")
        RA = main.tile([128, 9 * NR], F32, tag="RA")     # Rres f32
        RA16 = main.tile([128, 9 * NR], F16, tag="RA16")
        vm = main.tile([128, 9 * NR], F16, tag="vm")
        P2 = main.tile([128, 9 * 2 * NB], F32, tag="P2")
        P2e2 = main.tile([128, 9 * 2 * NB], F16, tag="P2e2")
        P4A = main.tile([128, 9 * NB], F32, tag="P4A")
        P4B = main.tile([128, 9 * NB], F32, tag="P4B")
        US = main.tile([128, 9 * 4 * NB], F16, tag="US")
        WS = main.tile([128, 9 * 2 * NB], F16, tag="WS")
        L4 = main.tile([128, 9 * 4 * NB], F16, tag="L4")
        T9a = main.tile([128, 9 * 4 * NB], F16, tag="T9a")
        T9b = main.tile([128, 9 * 4 * NB], F16, tag="T9b")
        T9af = main.tile([128, 18 * NB], F32, tag="T9af")
        T9bf = main.tile([128, 18 * NB], F32, tag="T9bf")
        Uloc = main.tile([128, 3 * W], F16, tag="Uloc")  # blocked u
        Uord = main.tile([128, 3 * W], F16, tag="Uord")  # ordered u
        Pall = main.tile([128, 3 * W], F32, tag="Pall")  # scanned (masked)
        Pmall = main.tile([128, 3 * W], F32, tag="Pmall")

        jplane_i = main.tile([128, W], I32, tag="jplane_i")
        jplane = main.tile([128, W], F32, tag="jplane")
        maskp = main.tile([128, W], F16, tag="maskp")
        thr = main.tile([128, 1], F32, tag="thr")
        Lsb = main.tile([BPC, 1], I32, tag="Lsb")
        Lf = main.tile([BPC, 1], F32, tag="Lf")
        Psb = main.tile([1, 6], F32, tag="Psb")
        kv = main.tile([1, 3], F32, tag="kv")
        Rv = main.tile([1, 3], F32, tag="Rv")
        NSC = 24
        vecs = main.tile([1, NSC], F32, tag="vecs")
        Vb = main.tile([128, NSC], F32, tag="Vb")
        zb1 = main.tile([1, 1], F32, tag="zb1")
        zb128 = main.tile([128, 1], F32, tag="zb128")
        warm = main.tile([1, 1], F32, tag="warm")
        ones16 = main.tile([128, 1], F16, tag="ones16")
        onesr = main.tile([128, NR], F16, tag="onesr")

        # selectors
        rowid_i = main.tile([128, 1], I32, tag="rowid_i")
        rowid = main.tile([128, 1], F32, tag="rowid")
        rowq = main.tile([128, 1], F32, tag="rowq")
        colw128 = main.tile([128, 128], I32, tag="colw128")   # i % 32
        selbq = main.tile([128, QN * 128], F16, tag="selbq")  # all-row gather
        selbqF = main.tile([128, QN * 128], F32, tag="selbqF")
        selbT = main.tile([BPC, 128], F32, tag="selbT")       # Lbc bcast

        # cross-quarter fixup (redundantly on all 128 rows, f32)
        Estack = main.tile([128, 36], F32, tag="Estack")
        Fstack = main.tile([128, 27], F32, tag="Fstack")
        Fbc = main.tile([128, 9], F32, tag="Fbc")
        Sg = main.tile([128, 12], F32, tag="Sg")
        Gv = main.tile([128, 9], F32, tag="Gv")
        cumst = main.tile([128, 9], F32, tag="cumst")
        FtP = main.tile([128, 9], F32, tag="FtP")
        FtPb = main.tile([128, 3], F32, tag="FtPb")
        Sall = main.tile([128, 3], F32, tag="Sall")
        Sall16 = main.tile([128, 3], F16, tag="Sall16")
        mt0 = main.tile([128, 9], F32, tag="mt0")
        mt1 = main.tile([128, 9], F32, tag="mt1")

        PSg = psum.tile([128, 36], F32, tag="PSg")
        PSp = psum.tile([128, 12], F32, tag="PSp")
        PSL = psum.tile([128, 1], F32, tag="PSL")

        V = nc.vector
        G = nc.gpsimd
        A = nc.scalar
        STT = nc.vector.scalar_tensor_tensor
        TS = nc.vector.tensor_scalar
        GTS = nc.gpsimd.tensor_scalar

        # ============ Phase A: DMAs + ACT warmup + setup ============
        nc.sync.dma_start(alpha[:], inp[:])
        nc.gpsimd.dma_start(Psb[:], par[:])
        nc.gpsimd.dma_start(Lsb[:], alen[:])
        V.memset(zb1[:], 0.0)
        V.memset(zb128[:], 0.0)
        # trigger both ACT table loads immediately (Copy set, Sin set)
        A.copy(warm[:], zb1[:])
        A.activation(warm[:], zb1[:], AF.Sin, bias=zb1[:])

        # ============ Phase B: trig (fp16 double-angle chains) ========
        # engines per type block: ACT does sin(a/8), sin(a/4), sq8;
        # chains on DVE (N, A) and Pool (C).
        def trig_block(t, E):
            TSx = V.tensor_scalar if E is V else G.tensor_scalar
            bs = slice(t * NR, (t + 1) * NR)
            avb = alpha[:, bs]
            s8, s4, sq, s2 = ts8[:, bs], ts4[:, bs], tsq[:, bs], ts2[:, bs]
            cab, sab = ca[:, bs], sa[:, bs]
            A.activation(s8, avb, AF.Sin, bias=zb128[:], scale=0.125)
            A.activation(s4, avb, AF.Sin, bias=zb128[:], scale=0.25)
            A.square(s8, s8)                                   # sq8
            TSx(cab, s8, -2.0, 1.0, op0=OP.mult, op1=OP.add)   # c4
            TSx(s4, s4, 2.0, None, op0=OP.mult)                # 2*s4
            E.tensor_mul(s2, s4, cab)                          # s2 = 2 s4 c4
            E.tensor_mul(sq, s4, s4)                           # (2 s4)^2
            TSx(cab, sq, -0.5, 1.0, op0=OP.mult, op1=OP.add)   # c2
            TSx(s2, s2, 2.0, None, op0=OP.mult)                # 2*s2
            E.tensor_mul(sab, s2, cab)                         # s1
            E.tensor_mul(sq, s2, s2)                           # (2 s2)^2
            TSx(cab, sq, -0.5, 1.0, op0=OP.mult, op1=OP.add)   # c1

        trig_block(0, V)
        trig_block(1, V)
        trig_block(2, G)

        # ============ Phase C: param scalars ============
        for t, idx in enumerate((5, 1, 3)):   # kappa: CA_C_N, C_N_CA, N_CA_C
            V.tensor_copy(kv[0:1, t:t + 1], Psb[0:1, idx:idx + 1])
        for t, idx in enumerate((4, 0, 2)):   # R: R_C_N, R_N_CA, R_CA_C
            V.tensor_copy(Rv[0:1, t:t + 1], Psb[0:1, idx:idx + 1])
        sk3 = main.tile([1, 3], F32, tag="sk3")
        ck3 = main.tile([1, 3], F32, tag="ck3")
        kvr = main.tile([1, 3], F32, tag="kvr")
        A.activation(sk3[:], kv[0:1, 0:3], AF.Sin, bias=zb1[:])
        A.activation(kvr[:], kv[0:1, 0:3], AF.Sin, bias=zb1[:], scale=0.5)
        A.square(kvr[:], kvr[:])
        V.tensor_scalar(ck3[:], kvr[:], -2.0, 1.0, op0=OP.mult, op1=OP.add)

        # scalar slots in vecs[1, NSC]:
        # 0:ckN 1:skN 2:ckA 3:skA 4:ckC 5:skC
        # 6:ckNckA 7:skNskA 8:ckNskA 9:skNckA
        # 10:nskNckA 11:nckNskA 12:nckN 13:nckA 14:nckC 15:nskA
        # 16:RNckN 17:RNskN 18:RCA 19:RC
        def vc(i):
            return vecs[0:1, i:i + 1]

        for t in range(3):
            V.tensor_copy(vc(2 * t), ck3[0:1, t:t + 1])
            V.tensor_copy(vc(2 * t + 1), sk3[0:1, t:t + 1])
        V.tensor_mul(vc(6), vc(0), vc(2))
        V.tensor_mul(vc(7), vc(1), vc(3))
        V.tensor_mul(vc(8), vc(0), vc(3))
        V.tensor_mul(vc(9), vc(1), vc(2))
        V.tensor_scalar_mul(vc(10), vc(9), -1.0)
        V.tensor_scalar_mul(vc(11), vc(8), -1.0)
        V.tensor_scalar_mul(vc(12), vc(0), -1.0)
        V.tensor_scalar_mul(vc(13), vc(2), -1.0)
        V.tensor_scalar_mul(vc(14), vc(4), -1.0)
        V.tensor_scalar_mul(vc(15), vc(3), -1.0)
        V.tensor_mul(vc(16), Rv[0:1, 0:1], vc(0))
        V.tensor_mul(vc(17), Rv[0:1, 0:1], vc(1))
        V.tensor_copy(vc(18), Rv[0:1, 1:2])
        V.tensor_copy(vc(19), Rv[0:1, 2:3])
        G.partition_broadcast(Vb[:], vecs[:])

        S = {}
        for i, nm in enumerate(("ckN", "skN", "ckA", "skA", "ckC", "skC",
                                "ckNckA", "skNskA", "ckNskA", "skNckA",
                                "nskNckA", "nckNskA", "nckN", "nckA",
                                "nckC", "nskA", "RNckN", "RNskN",
                                "RCA", "RC")):
            S[nm] = Vb[:, i:i + 1]

        # ============ Phase A2: setup on Pool (post-DMA issue) ========
        G.iota(rowid_i[:], [[0, 1]], channel_multiplier=1)
        V.tensor_copy(rowid[:], rowid_i[:])
        # colw128[p, i] = i % 32
        G.iota(colw128[:], [[0, QN], [1, BPC]], channel_multiplier=0)
        # selbq block q: (p == 32q + i%32)  <=>  (i%32 == p - 32q)
        for q in range(QN):
            TS(rowq[:], rowid[:], float(q * BPC), None, op0=OP.subtract)
            TS(selbq[:, q * 128:(q + 1) * 128], colw128[:], rowq[:, 0:1],
               None, op0=OP.is_equal)
            GTS(selbqF[:, q * 128:(q + 1) * 128], colw128[:], rowq[:, 0:1],
                None, op0=OP.is_equal)
        # selbT[b, i] = (i % 32 == b)
        TS(selbT[0:BPC, :], colw128[0:BPC, :], rowid[0:BPC, 0:1], None,
           op0=OP.is_equal)
        G.memset(ones16[:], 1.0)
        G.memset(onesr[:], 1.0)
        G.memset(L4[:], 0.0)

        # mask (early): Lbc broadcast via PE, thr, maskp
        V.tensor_copy(Lf[:], Lsb[:])
        nc.tensor.matmul(PSL[:, 0:1], selbT[0:BPC, :], Lf[:, 0:1],
                         start=True, stop=True)
        G.iota(jplane_i[:], [[1, W]], channel_multiplier=0)
        V.tensor_copy(jplane[:], jplane_i[:])
        for q in range(QN):
            TS(thr[q * BPC:(q + 1) * BPC, 0:1],
               PSL[q * BPC:(q + 1) * BPC, 0:1],
               3.0, float(q * W), op0=OP.mult, op1=OP.subtract)
        TS(maskp[:], jplane[:], thr[:, 0:1], None, op0=OP.is_lt)

        # ============ Phase D: C1 = B_N @ B_CA (fp16) ============
        cN, sN = ca[:, 0:NR], sa[:, 0:NR]
        cA, sA = ca[:, NR:2 * NR], sa[:, NR:2 * NR]
        cC, sC = ca[:, 2 * NR:3 * NR], sa[:, 2 * NR:3 * NR]

        def blk(t, e, lo=0, hi=NR):
            return t[:, e * NR + lo:e * NR + hi]

        pp1 = PP[:, 0 * NR:1 * NR]
        pp2 = PP[:, 1 * NR:2 * NR]
        pp3 = PP[:, 2 * NR:3 * NR]
        pp4 = PP[:, 3 * NR:4 * NR]
        V.tensor_mul(pp1, cN, cA)
        V.tensor_mul(pp2, sN, sA)
        G.tensor_mul(pp3, cN, sA)
        G.tensor_mul(pp4, sN, cA)
        c1 = C1[:]
        TS(blk(c1, 0), cA, S["skNskA"], S["ckNckA"],
           op0=OP.mult, op1=OP.add)                       # C1_00
        TS(blk(c1, 1), cA, S["nskNckA"], S["ckNskA"],
           op0=OP.mult, op1=OP.add)                       # C1_01
        V.tensor_scalar_mul(blk(c1, 2), sA, S["skN"])     # C1_02
        x = blk(c1, 3)                                    # C1_10
        A.mul(x, cN, S["skNckA"])
        STT(x, pp1, S["nckNskA"], x, op0=OP.mult, op1=OP.add)
        STT(x, pp2, S["skA"], x, op0=OP.mult, op1=OP.add)
        x = blk(c1, 4)                                    # C1_11
        A.mul(x, cN, S["skNskA"])
        STT(x, pp1, S["ckNckA"], x, op0=OP.mult, op1=OP.add)
        STT(x, pp2, S["nckA"], x, op0=OP.mult, op1=OP.add)
        STT(blk(c1, 5), pp3, S["nckN"], pp4,
            op0=OP.mult, op1=OP.subtract)                 # C1_12
        x = blk(c1, 6)                                    # C1_20
        A.mul(x, sN, S["skNckA"])
        STT(x, pp4, S["nckNskA"], x, op0=OP.mult, op1=OP.add)
        STT(x, pp3, S["nskA"], x, op0=OP.mult, op1=OP.add)
        x = blk(c1, 7)                                    # C1_21
        A.mul(x, sN, S["skNskA"])
        STT(x, pp4, S["ckNckA"], x, op0=OP.mult, op1=OP.add)
        STT(x, pp3, S["ckA"], x, op0=OP.mult, op1=OP.add)
        STT(blk(c1, 8), pp2, S["nckN"], pp1,
            op0=OP.mult, op1=OP.add)                      # C1_22

        # residue-0 of q=0 rows: C1 := B_CA(0) (alpha_CA(0)=0)
        r0s = slice(0, BPC)
        o1 = ones16[r0s, 0:1]
        V.tensor_scalar_mul(c1[r0s, 0 * NR:0 * NR + 1], o1, S["ckA"][r0s])
        V.tensor_scalar_mul(c1[r0s, 1 * NR:1 * NR + 1], o1, S["skA"][r0s])
        V.memset(c1[r0s, 2 * NR:2 * NR + 1], 0.0)
        V.tensor_scalar_mul(c1[r0s, 3 * NR:3 * NR + 1], o1, S["skA"][r0s])
        V.tensor_scalar_mul(c1[r0s, 4 * NR:4 * NR + 1], o1, S["nckA"][r0s])
        V.memset(c1[r0s, 5 * NR:5 * NR + 1], 0.0)
        V.memset(c1[r0s, 6 * NR:6 * NR + 1], 0.0)
        V.memset(c1[r0s, 7 * NR:7 * NR + 1], 0.0)
        V.memset(c1[r0s, 8 * NR:8 * NR + 1], -1.0)

        # ============ Phase E: Rres = C1 @ B_C -> RA (fp16), vm ========
        ra = RA[:]
        for i in range(3):
            (V if i != 2 else G).tensor_mul(blk(QQ[:], i),
                                            blk(c1, 3 * i + 1), cC)
            (G if i != 2 else V).tensor_mul(blk(QQ[:], 3 + i),
                                            blk(c1, 3 * i + 2), sC)
        for i in range(3):
            q1i = blk(QQ[:], i)
            q2i = blk(QQ[:], 3 + i)
            x = blk(ra, 3 * i + 0)
            A.mul(x, blk(c1, 3 * i + 0), S["ckC"])
            STT(x, q1i, S["skC"], x, op0=OP.mult, op1=OP.add)
            STT(x, q2i, S["skC"], x, op0=OP.mult, op1=OP.add)
            x = blk(ra, 3 * i + 1)
            A.mul(x, blk(c1, 3 * i + 0), S["skC"])
            STT(x, q1i, S["nckC"], x, op0=OP.mult, op1=OP.add)
            STT(x, q2i, S["nckC"], x, op0=OP.mult, op1=OP.add)
            t1 = tsq[:, i * NR:(i + 1) * NR]
            t2 = ts2[:, i * NR:(i + 1) * NR]
            V.tensor_mul(t1, blk(c1, 3 * i + 1), sC)
            G.tensor_mul(t2, blk(c1, 3 * i + 2), cC)
            V.tensor_sub(blk(ra, 3 * i + 2), t1, t2)

        # v-vectors (fp16): vm plane p = 3*vec + coord
        vmv = vm[:]
        V.tensor_scalar_mul(blk(vmv, 0), onesr[:], S["RNckN"])
        A.mul(blk(vmv, 1), cN, S["RNskN"])
        A.mul(blk(vmv, 2), sN, S["RNskN"])
        for i in range(3):
            A.mul(blk(vmv, 3 + i), blk(c1, 3 * i + 0), S["RCA"])
            A.mul(blk(vmv, 6 + i), blk(ra, 3 * i + 0), S["RC"])

        # ============ Phase F: scan machinery (fp16) ============
        # generic fused 3-term matmul group, col-split DVE/Pool
        def fused(Lf_, Rf_, Of_, Tf_, n, split=0.85, ta=None, tb=None):
            ta = T9a if ta is None else ta
            tb = T9b if tb is None else tb
            cut = min(n, max(0, int(n * split)))
            segs = []
            if cut > 0:
                segs.append((V, 0, cut))
            if cut < n:
                segs.append((G, cut, n))
            for E, c0, c1_ in segs:
                E.tensor_mul(Tf_(ta, c0, c1_), Lf_(0, c0, c1_),
                             Rf_(0, c0, c1_))
                E.tensor_mul(Tf_(tb, c0, c1_), Lf_(1, c0, c1_),
                             Rf_(1, c0, c1_))
                E.tensor_add(Tf_(ta, c0, c1_), Tf_(ta, c0, c1_),
                             Tf_(tb, c0, c1_))
                E.tensor_mul(Tf_(tb, c0, c1_), Lf_(2, c0, c1_),
                             Rf_(2, c0, c1_))
                E.tensor_add(Of_(c0, c1_), Tf_(ta, c0, c1_),
                             Tf_(tb, c0, c1_))

        # fp16 shadow of Rres for the expansion products
        V.tensor_copy(RA16[:], RA[:])

        # --- pair: P2[b, j] = RA[blk 2b, j] @ RA[blk 2b+1, j]
        for b in range(2):
            base = 64 * b

            def PL(k, c0, c1_, base=base):
                return _ap(ra, k * NR + base + c0,
                           [[3 * NR, 3], [0, 3], [1, c1_ - c0]])

            def PR(k, c0, c1_, base=base):
                return _ap(ra, 3 * k * NR + base + NB + c0,
                           [[0, 3], [NR, 3], [1, c1_ - c0]])

            def PO(c0, c1_, base=32 * b):
                return _ap(P2[:], base + c0,
                           [[192, 3], [64, 3], [1, c1_ - c0]])

            def PT(t, c0, c1_, base=288 * b):
                return _ap(t[:], base + c0, [[96, 3], [32, 3], [1, c1_ - c0]])

            fused(PL, PR, PO, PT, NB, split=0.7, ta=T9af, tb=T9bf)

        # --- Wodd: b=0 -> US cols 32:64 ; b=1 -> WS cols 32:64
        for b, (dst, dstride) in enumerate(((US, 128), (WS, 64))):
            base = 64 * b

            def WL(k, c0, c1_, base=base):
                return _ap(RA16[:], k * NR + base + c0,
                           [[3 * NR, 3], [0, 3], [1, c1_ - c0]])

            def WR(k, c0, c1_, base=base):
                return _ap(vmv, k * NR + base + NB + c0,
                           [[0, 3], [3 * NR, 3], [1, c1_ - c0]])

            def WO(c0, c1_, dst=dst, ds=dstride):
                return _ap(dst[:], NB + c0,
                           [[ds, 3], [3 * ds, 3], [1, c1_ - c0]])

            def WT(t, c0, c1_, base=576 + 288 * b):
                return _ap(t[:], base + c0,
                           [[32, 3], [96, 3], [1, c1_ - c0]])

            fused(WL, WR, WO, WT, NB, split=0.84)

        # --- quad: P4[j] = P2[b0, j] @ P2[b1, j]
        def QL(k, c0, c1_):
            return _ap(P2[:], k * 64 + c0, [[192, 3], [0, 3], [1, c1_ - c0]])

        def QR(k, c0, c1_):
            return _ap(P2[:], 3 * k * 64 + NB + c0,
                       [[0, 3], [64, 3], [1, c1_ - c0]])

        def QO(c0, c1_):
            return _ap(P4A[:], c0, [[96, 3], [32, 3], [1, c1_ - c0]])

        def QT(t, c0, c1_):
            return _ap(t[:], c0, [[96, 3], [32, 3], [1, c1_ - c0]])

        fused(QL, QR, QO, QT, NB, split=0.7, ta=T9af, tb=T9bf)

        # --- P2e2: replicate P2 even block x2 ; vm m2 -> WS ; vm m0 -> US
        V.tensor_copy(_ap(P2e2[:], 0, [[64, 9], [32, 2], [1, 32]]),
                      _ap(P2[:], 0, [[64, 9], [0, 2], [1, 32]]))
        V.tensor_copy(_ap(WS[:], 0, [[64, 9], [1, 32]]),
                      _ap(vmv, 64, [[NR, 9], [1, 32]]))
        V.tensor_copy(_ap(US[:], 0, [[128, 9], [1, 32]]),
                      _ap(vmv, 0, [[NR, 9], [1, 32]]))

        # --- W2 merged: out US cols 64:128 = P2even @ WS
        def W2L(k, c0, c1_):
            return _ap(P2e2[:], k * 64 + c0,
                       [[192, 3], [0, 3], [1, c1_ - c0]])

        def W2R(k, c0, c1_):
            return _ap(WS[:], k * 64 + c0, [[0, 3], [192, 3], [1, c1_ - c0]])

        def W2O(c0, c1_):
            return _ap(US[:], 64 + c0, [[128, 3], [384, 3], [1, c1_ - c0]])

        def W2T(t, c0, c1_):
            return _ap(t[:], 288 + c0, [[64, 3], [192, 3], [1, c1_ - c0]])

        fused(W2L, W2R, W2O, W2T, 2 * NB, split=0.84)

        # --- Hillis-Steele over 32 quads
        def hs_step(srcb, dstb, s):
            n = NB - s
            sv = srcb.rearrange("p (e j) -> p e j", e=9)
            dv = dstb.rearrange("p (e j) -> p e j", e=9)
            A.copy(dv[:, :, 0:s], sv[:, :, 0:s])

            def HL(k, c0, c1_):
                return _ap(srcb, k * NB + c0,
                           [[96, 3], [0, 3], [1, c1_ - c0]])

            def HR(k, c0, c1_):
                return _ap(srcb, 3 * k * NB + s + c0,
                           [[0, 3], [32, 3], [1, c1_ - c0]])

            def HO(c0, c1_):
                return _ap(dstb, s + c0, [[96, 3], [32, 3], [1, c1_ - c0]])

            def HT(t, c0, c1_):
                return _ap(t[:], c0, [[96, 3], [32, 3], [1, c1_ - c0]])

            fused(HL, HR, HO, HT, n, split=0.7, ta=T9af, tb=T9bf)

        bufs = [P4A, P4B]
        for step in range(5):
            hs_step(bufs[step % 2][:], bufs[(step + 1) % 2][:], 1 << step)
        Rscan = bufs[1][:]    # P4B

        # --- rotation fixup: E gathered to ALL rows, F chain, slice Fbc ---
        for q in range(QN):
            nc.tensor.matmul(
                PSg[:, q * 9:(q + 1) * 9],
                selbqF[:, q * 128:(q + 1) * 128],
                _ap(Rscan, NB - 1, [[NB, 9]]), start=True, stop=True)
        V.tensor_copy(Estack[:, 0:36], PSg[:, 0:36])
        V.tensor_copy(Fstack[:, 0:9], Estack[:, 0:9])
        fs = Fstack[:]
        es = Estack[:]

        def ap2(base_ap, off, dims):
            return _ap(base_ap, off, dims)

        for q in (1, 2):
            FL = lambda k: ap2(fs, (q - 1) * 9 + k, [[3, 3], [0, 3]])
            ER = lambda k: ap2(es, q * 9 + 3 * k, [[0, 3], [1, 3]])
            MT = lambda t: ap2(t[:], 0, [[3, 3], [1, 3]])
            FO = ap2(fs, q * 9, [[3, 3], [1, 3]])
            V.tensor_mul(MT(mt0), FL(0), ER(0))
            V.tensor_mul(MT(mt1), FL(1), ER(1))
            V.tensor_add(MT(mt0), MT(mt0), MT(mt1))
            V.tensor_mul(MT(mt1), FL(2), ER(2))
            V.tensor_add(FO, MT(mt0), MT(mt1))
        # Fbc: rows 0:32 identity; quarter q rows take F_q slice
        V.memset(Fbc[0:BPC, 0:9], 0.0)
        V.memset(bass.AP(Fbc[:].tensor, Fbc[:].offset,
                         [[Fbc[:].ap[0][0], BPC], [4, 3]]), 1.0)
        for q in (1, 2, 3):
            V.tensor_copy(Fbc[q * BPC:(q + 1) * BPC, 0:9],
                          Fstack[q * BPC:(q + 1) * BPC,
                                 (q - 1) * 9:q * 9])

        # --- L4: shifted replicate of Rscan ---
        V.tensor_copy(_ap(L4[:], 1, [[128, 9], [32, 4], [1, NB - 1]]),
                      _ap(Rscan, 0, [[NB, 9], [0, 4], [1, NB - 1]]))

        # --- U merged: Uloc[c][j*128 + g*32 + r] = L4 @ US
        def UL(k, c0, c1_):
            return _ap(L4[:], k * 128 + c0,
                       [[384, 3], [0, 3], [1, c1_ - c0]])

        def UR(k, c0, c1_):
            return _ap(US[:], k * 128 + c0,
                       [[0, 3], [384, 3], [1, c1_ - c0]])

        def UO(c0, c1_):
            return _ap(Uloc[:], c0, [[384, 3], [128, 3], [1, c1_ - c0]])

        def UT(t, c0, c1_):
            return _ap(t[:], c0, [[384, 3], [128, 3], [1, c1_ - c0]])

        fused(UL, UR, UO, UT, 4 * NB, split=0.84)

        # seeds: r=0 of each g-block: identity prefix -> u = US col g*32
        V.tensor_copy(_ap(Uloc[:], 0, [[384, 3], [128, 3], [32, 4]]),
                      _ap(US[:], 0, [[128, 3], [384, 3], [32, 4]]))
        # atom 0 of whole chain (q=0 rows) at origin
        V.memset(bass.AP(Uloc[:].tensor, Uloc[:].offset,
                         [[Uloc[:].ap[0][0], BPC], [384, 3]]), 0.0)

        # ============ Phase G: tail ============
        # quarter sums S (blocked u, order-independent) -> Sall [128,3]
        for c in range(3):
            V.tensor_reduce(Sall[:, c:c + 1], Uloc[:, c * W:(c + 1) * W],
                            axis=mybir.AxisListType.X, op=OP.add)
        V.tensor_copy(Sall16[:], Sall[:])
        # gather S per quarter to ALL rows -> Sg [128, 12]
        for q in range(QN):
            nc.tensor.matmul(
                PSp[:, q * 3:(q + 1) * 3],
                selbq[:, q * 128:(q + 1) * 128],
                Sall16[:, 0:3], start=True, stop=True)
        V.tensor_copy(Sg[:, 0:12], PSp[:, 0:12])
        # G_q = F_q @ S_q (q=0: F=I)
        gv = Gv[:]
        sg = Sg[:]
        V.tensor_copy(Gv[:, 0:3], Sg[:, 0:3])
        for q in (1, 2):
            FL = lambda k: ap2(fs, (q - 1) * 9 + k, [[3, 3]])
            SR = lambda k: ap2(sg, q * 3 + k, [[0, 3]])
            M3 = lambda t: ap2(t[:], 0, [[1, 3]])
            GO = ap2(gv, q * 3, [[1, 3]])
            V.tensor_mul(M3(mt0), FL(0), SR(0))
            V.tensor_mul(M3(mt1), FL(1), SR(1))
            V.tensor_add(M3(mt0), M3(mt0), M3(mt1))
            V.tensor_mul(M3(mt1), FL(2), SR(2))
            V.tensor_add(GO, M3(mt0), M3(mt1))
        # cumulative: Pinc_q for q=1..3 at cumst cols (q-1)*3
        V.tensor_copy(cumst[:, 0:3], Gv[:, 0:3])
        V.tensor_add(cumst[:, 3:6], cumst[:, 0:3], Gv[:, 3:6])
        V.tensor_add(cumst[:, 6:9], cumst[:, 3:6], Gv[:, 6:9])
        # FtP_q = F_q^T @ Pinc_q  (transpose via swapped strides)
        ftp = FtP[:]
        cs = cumst[:]
        for q in (1, 2, 3):
            FLT = lambda k: ap2(fs, (q - 1) * 9 + 3 * k, [[1, 3]])
            PR2 = lambda k: ap2(cs, (q - 1) * 3 + k, [[0, 3]])
            M3 = lambda t: ap2(t[:], 0, [[1, 3]])
            FO2 = ap2(ftp, (q - 1) * 3, [[1, 3]])
            V.tensor_mul(M3(mt0), FLT(0), PR2(0))
            V.tensor_mul(M3(mt1), FLT(1), PR2(1))
            V.tensor_add(M3(mt0), M3(mt0), M3(mt1))
            V.tensor_mul(M3(mt1), FLT(2), PR2(2))
            V.tensor_add(FO2, M3(mt0), M3(mt1))
        # FtPb: rows 0:32 zero; quarter q rows take FtP_q slice
        V.memset(FtPb[0:BPC, 0:3], 0.0)
        for q in (1, 2, 3):
            V.tensor_copy(FtPb[q * BPC:(q + 1) * BPC, 0:3],
                          FtP[q * BPC:(q + 1) * BPC, (q - 1) * 3:q * 3])

        # reorder u blocked -> ordered (fp16)
        for c, E in ((0, V), (1, G), (2, V)):
            E.tensor_copy(
                _ap(Uord[:], c * W, [[12, 32], [3, 4], [1, 3]]),
                _ap(Uloc[:], c * W, [[1, 32], [32, 4], [128, 3]]))
        # masked scan seeded with FtP
        for c, E in ((0, V), (1, V), (2, V)):
            E.tensor_tensor_scan(
                Pall[:, c * W:(c + 1) * W], Uord[:, c * W:(c + 1) * W],
                maskp[:], FtPb[:, c:c + 1], op0=OP.add, op1=OP.mult)
        # frame fix + store: out_c = F_c0*px + F_c1*py + F_c2*pz
        for c in range(3):
            x = Pmall[:, c * W:(c + 1) * W]
            A.mul(x, Pall[:, 0:W], Fbc[:, 3 * c + 0:3 * c + 1])
            STT(x, Pall[:, W:2 * W], Fbc[:, 3 * c + 1:3 * c + 2], x,
                op0=OP.mult, op1=OP.add)
            STT(x, Pall[:, 2 * W:3 * W], Fbc[:, 3 * c + 2:3 * c + 3], x,
                op0=OP.mult, op1=OP.add)
            (nc.sync if c != 1 else nc.scalar).dma_start(
                out[:, c * W:(c + 1) * W], x)


def _prep_alpha(input):
    # alphaN[r]=psi[r-1], alphaCA[r]=omega[r-1] (0 at r=0), alphaC[r]=phi[r];
    # then mod-4 block permutation within each 128-residue quarter.
    phi, psi, om = input[:, 0], input[:, 1], input[:, 2]
    z1 = np.zeros((input.shape[0], 1), np.float32)
    aN = np.concatenate([z1, psi[:, :-1]], axis=1)
    aCA = np.concatenate([z1, om[:, :-1]], axis=1)
    alpha = np.stack([aN, aCA, phi], axis=1)          # [B, 3, 512]
    alpha = alpha.reshape(-1, 3, QN, NR)
    perm = np.arange(NR).reshape(NB, 4).T.reshape(-1)  # sigma^-1: col->r
    alpha = alpha[..., perm]                           # blocked columns
    return alpha.transpose(0, 2, 1, 3)                 # [B, QN, 3, NR]


def _shard_alpha(alpha, i):
    sl = slice(i * BPC, (i + 1) * BPC)
    return np.ascontiguousarray(
        alpha[sl].transpose(1, 0, 2, 3).reshape(QN * BPC, 3 * NR))


def _get_nc():
    if "nc" not in _CACHE:
        _CACHE["nc"] = _build_graph()
    return _CACHE["nc"]


def kernel(input, param, angles_length, trace=False):
    input = np.ascontiguousarray(input, dtype=np.float32)
    param = np.ascontiguousarray(param, dtype=np.float32)
    angles_length = np.ascontiguousarray(angles_length, dtype=np.int32)
    nc = _get_nc()
    alpha = _prep_alpha(input)
    in_maps = []
    for i in range(NCORES):
        sl = slice(i * BPC, (i + 1) * BPC)
        in_maps.append({
            "input": _shard_alpha(alpha, i),
            "param": param,
            "angles_length": angles_length[sl],
        })
    res = run_bass_kernel_spmd(nc, in_maps, core_ids=list(range(NCORES)),
                               trace=trace)
    outs = []
    for i in range(NCORES):
        r = res.results[i]["out"]          # [(q,b), (c,j)]
        r = r.reshape(QN, BPC, 3, W)
        r = np.transpose(r, (1, 0, 3, 2)).reshape(BPC, 3 * QN * W)
        outs.append(r)
    full = np.concatenate(outs, axis=0).astype(np.float32)
    if trace:
        kernel._last_exec_ns = res.exec_time_ns
    return full


kernel._last_exec_ns = None


# revision 32
# speedup vs baseline: 1.0927x; 1.0238x over previous
"""Trainium2 Bass kernel for Angles2Backbone (NeRF chain forward).

Full inputs: input [256,3,512] f32, param [6] f32, angles_length [256] i32.
Output: [256, 4608] f32  (coords of 1536 backbone atoms x 3, masked).

Sharding: pure data parallel over batch - 32 proteins per core x 8 cores.

Per-core algorithm (v3: fp16 scan machinery, mod-4 blocked residue layout):
  - 128 partitions = (quarter q)*32 + protein b; each row owns 128 residues.
  - Residue r of a quarter lives at column sigma(r) = 32*(r%4) + r//4
    (host-side permutation), so every pair/quad/expansion op reads and
    writes stride-1 column blocks -> DVE 2x fp16 mode throughout.
  - Trig via sin LUT at a/8, a/4 + double-angle chains (fp16).
  - Pre-pass builds per-residue rotation Rres (fp16) from scalar-folded
    bilinear terms; v-vectors (per-atom translations) in fp16.
  - pair (mod-4 blocks) -> P2, quad -> P4, Hillis-Steele over 32 quads.
  - Vector expansions Wodd/W2 written straight into a fused source tensor
    US so the superquad expansion is 5 big ops against a replicated,
    shifted prefix tensor L4.
  - Positions: blocked u -> reorder copy -> masked tensor_tensor_scan
    seeded with F^T.Pinc (cross-quarter fixup folded into the scan),
    then frame rotation F and store.
  - Cross-partition moves via PE matmuls only (no SBUF-SBUF DMA).
"""

import sys

sys.path.insert(0, "/opt/trn_rl_repo")

import numpy as np
import concourse.bass as bass
import concourse.bacc as bacc
import concourse.mybir as mybir
from concourse import tile
from concourse.bass_utils import run_bass_kernel_spmd

F32 = mybir.dt.float32
F16 = mybir.dt.float16
I32 = mybir.dt.int32
AF = mybir.ActivationFunctionType
OP = mybir.AluOpType

NCORES = 8
BPC = 32          # proteins per core
L = 512           # residues per protein
QN = 4            # chain quarters per protein (partition groups)
W = 384           # atoms per quarter
NR = 128          # residues per quarter
NB = 32           # columns per mod-4 block
PI = float(np.pi)

_CACHE = {}


def _build_graph():
    nc = bacc.Bacc("TRN2", target_bir_lowering=False, debug=False,
                   num_devices=NCORES)
    inp = nc.dram_tensor("input", [QN * BPC, 3 * NR], F32,
                         kind="ExternalInput").ap()
    par = nc.dram_tensor("param", [6], F32, kind="ExternalInput").ap()
    alen = nc.dram_tensor("angles_length", [BPC], I32,
                          kind="ExternalInput").ap()
    out = nc.dram_tensor("out", [QN * BPC, 3 * W], F32,
                         kind="ExternalOutput").ap()
    with tile.TileContext(nc) as tc:
        _emit(nc, tc, inp, par, alen, out)
    nc.compile()
    return nc


def _ap(base_ap, off, dims):
    return bass.AP(base_ap.tensor, base_ap.offset + off,
                   [list(base_ap.ap[0])] + [list(d) for d in dims])


def _emit(nc, tc, inp, par, alen, out):
    import contextlib
    ctx = contextlib.ExitStack()
    with ctx:
        main = ctx.enter_context(tc.tile_pool(name="main", bufs=1))
        psum = ctx.enter_context(tc.tile_pool(name="psum", bufs=1,
                                              space="PSUM"))

        # ---------------- tiles ----------------
        alpha = main.tile([128, 3 * NR], F32, tag="alpha")
        ca = main.tile([128, 3 * NR], F32, tag="ca")
        sa = main.tile([128, 3 * NR], F32, tag="sa")
        # trig scratch (per type block)
        ts8 = main.tile([128, 3 * NR], F32, tag="ts8")   # s8 then sq8
        ts4 = main.tile([128, 3 * NR], F32, tag="ts4")   # s4 then s4*2
        tsq = main.tile([128, 3 * NR], F32, tag="tsq")   # squares
        ts2 = main.tile([128, 3 * NR], F32, tag="ts2")   # s2 then s2*2

        PP = main.tile([128, 4 * NR], F32, tag="PP")
        C1 = main.tile([128, 9 * NR], F32, tag="C1")
        QQ = main.tile([128, 6 * NR], F32, tag="QQ")
        RA = main.tile([128, 9 * NR], F32, tag="RA")     # Rres f32
        RA16 = main.tile([128, 9 * NR], F16, tag="RA16")
        vm = main.tile([128, 9 * NR], F16, tag="vm")
        P2 = main.tile([128, 9 * 2 * NB], F16, tag="P2")
        P2e2 = main.tile([128, 9 * 2 * NB], F16, tag="P2e2")
        P4A = main.tile([128, 9 * NB], F16, tag="P4A")
        P4B = main.tile([128, 9 * NB], F16, tag="P4B")
        US = main.tile([128, 9 * 4 * NB], F16, tag="US")
        WS = main.tile([128, 9 * 2 * NB], F16, tag="WS")
        L4 = main.tile([128, 9 * 4 * NB], F16, tag="L4")
        T9a = main.tile([128, 9 * 4 * NB], F16, tag="T9a")
        T9b = main.tile([128, 9 * 4 * NB], F16, tag="T9b")
        T9c = main.tile([128, 9 * 4 * NB], F16, tag="T9c")
        T9af = main.tile([128, 18 * NB], F32, tag="T9af")
        T9bf = main.tile([128, 18 * NB], F32, tag="T9bf")
        T9cf = main.tile([128, 18 * NB], F32, tag="T9cf")
        Uloc = main.tile([128, 3 * W], F16, tag="Uloc")  # blocked u
        Uord = main.tile([128, 3 * W], F16, tag="Uord")  # ordered u
        Pall = main.tile([128, 3 * W], F32, tag="Pall")  # scanned (masked)
        Pmall = main.tile([128, 3 * W], F32, tag="Pmall")

        jplane_i = main.tile([128, W], I32, tag="jplane_i")
        jplane = main.tile([128, W], F32, tag="jplane")
        maskp = main.tile([128, W], F16, tag="maskp")
        thr = main.tile([128, 1], F32, tag="thr")
        Lsb = main.tile([BPC, 1], I32, tag="Lsb")
        Lf = main.tile([BPC, 1], F32, tag="Lf")
        Psb = main.tile([1, 6], F32, tag="Psb")
        kv = main.tile([1, 3], F32, tag="kv")
        Rv = main.tile([1, 3], F32, tag="Rv")
        NSC = 24
        vecs = main.tile([1, NSC], F32, tag="vecs")
        Vb = main.tile([128, NSC], F32, tag="Vb")
        zb1 = main.tile([1, 1], F32, tag="zb1")
        zb128 = main.tile([128, 1], F32, tag="zb128")
        warm = main.tile([1, 1], F32, tag="warm")
        ones16 = main.tile([128, 1], F16, tag="ones16")
        onesr = main.tile([128, NR], F16, tag="onesr")

        # selectors
        rowid_i = main.tile([128, 1], I32, tag="rowid_i")
        rowid = main.tile([128, 1], F32, tag="rowid")
        rowq = main.tile([128, 1], F32, tag="rowq")
        colw128 = main.tile([128, 128], I32, tag="colw128")   # i % 32
        selbq = main.tile([128, QN * 128], F16, tag="selbq")  # all-row gather
        selbqF = main.tile([128, QN * 128], F32, tag="selbqF")
        selbT = main.tile([BPC, 128], F32, tag="selbT")       # Lbc bcast

        # cross-quarter fixup (redundantly on all 128 rows, f32)
        Estack = main.tile([128, 36], F32, tag="Estack")
        Fstack = main.tile([128, 27], F32, tag="Fstack")
        Fbc = main.tile([128, 9], F32, tag="Fbc")
        Sg = main.tile([128, 12], F32, tag="Sg")
        Gv = main.tile([128, 9], F32, tag="Gv")
        cumst = main.tile([128, 9], F32, tag="cumst")
        FtP = main.tile([128, 9], F32, tag="FtP")
        FtPb = main.tile([128, 3], F32, tag="FtPb")
        Sall = main.tile([128, 3], F32, tag="Sall")
        Sall16 = main.tile([128, 3], F16, tag="Sall16")
        mt0 = main.tile([128, 9], F32, tag="mt0")
        mt1 = main.tile([128, 9], F32, tag="mt1")

        PSg = psum.tile([128, 36], F32, tag="PSg")
        PSp = psum.tile([128, 12], F32, tag="PSp")
        PSL = psum.tile([128, 1], F32, tag="PSL")

        V = nc.vector
        G = nc.gpsimd
        A = nc.scalar
        STT = nc.vector.scalar_tensor_tensor
        TS = nc.vector.tensor_scalar
        GTS = nc.gpsimd.tensor_scalar

        # ============ Phase A: DMAs + ACT warmup + setup ============
        nc.sync.dma_start(alpha[:], inp[:])
        nc.gpsimd.dma_start(Psb[:], par[:])
        nc.gpsimd.dma_start(Lsb[:], alen[:])
        V.memset(zb1[:], 0.0)
        V.memset(zb128[:], 0.0)
        # trigger the Sin table load immediately (Copy set loads after sins)
        A.activation(warm[:], zb1[:], AF.Sin, bias=zb1[:])

        G.memset(ones16[:], 1.0)
        G.memset(onesr[:], 1.0)

        # ============ Phase B: trig (fp16 double-angle chains) ========
        # engines per type block: ACT does sin(a/8), sin(a/4), sq8;
        # chains on DVE (N, A) and Pool (C).
        def trig_sins(t):
            bs = slice(t * NR, (t + 1) * NR)
            avb = alpha[:, bs]
            A.activation(ts8[:, bs], avb, AF.Sin, bias=zb128[:], scale=0.125)
            A.activation(ts4[:, bs], avb, AF.Sin, bias=zb128[:], scale=0.25)

        def trig_chain(t, E):
            TSx = V.tensor_scalar if E is V else G.tensor_scalar
            bs = slice(t * NR, (t + 1) * NR)
            s8, s4, sq, s2 = ts8[:, bs], ts4[:, bs], tsq[:, bs], ts2[:, bs]
            cab, sab = ca[:, bs], sa[:, bs]
            E.tensor_mul(sq, s8, s8)                           # sin^2(a/8)
            TSx(cab, sq, -2.0, 1.0, op0=OP.mult, op1=OP.add)   # c4
            TSx(s4, s4, 2.0, None, op0=OP.mult)                # 2*s4
            E.tensor_mul(s2, s4, cab)                          # s2 = 2 s4 c4
            E.tensor_mul(sq, s4, s4)                           # (2 s4)^2
            TSx(cab, sq, -0.5, 1.0, op0=OP.mult, op1=OP.add)   # c2
            TSx(s2, s2, 2.0, None, op0=OP.mult)                # 2*s2
            E.tensor_mul(sab, s2, cab)                         # s1
            E.tensor_mul(sq, s2, s2)                           # (2 s2)^2
            TSx(cab, sq, -0.5, 1.0, op0=OP.mult, op1=OP.add)   # c1

        for t in range(3):
            trig_sins(t)
        trig_chain(0, V)
        trig_chain(1, V)
        trig_chain(2, G)

        # ============ Phase C: param scalars ============
        for t, idx in enumerate((5, 1, 3)):   # kappa: CA_C_N, C_N_CA, N_CA_C
            V.tensor_copy(kv[0:1, t:t + 1], Psb[0:1, idx:idx + 1])
        for t, idx in enumerate((4, 0, 2)):   # R: R_C_N, R_N_CA, R_CA_C
            V.tensor_copy(Rv[0:1, t:t + 1], Psb[0:1, idx:idx + 1])
        sk3 = main.tile([1, 3], F32, tag="sk3")
        ck3 = main.tile([1, 3], F32, tag="ck3")
        kvr = main.tile([1, 3], F32, tag="kvr")
        A.activation(sk3[:], kv[0:1, 0:3], AF.Sin, bias=zb1[:])
        A.activation(kvr[:], kv[0:1, 0:3], AF.Sin, bias=zb1[:], scale=0.5)
        A.copy(warm[:], zb1[:])     # Copy-set LUT load, off critical path
        V.tensor_mul(kvr[:], kvr[:], kvr[:])
        V.tensor_scalar(ck3[:], kvr[:], -2.0, 1.0, op0=OP.mult, op1=OP.add)

        # scalar slots in vecs[1, NSC]:
        # 0:ckN 1:skN 2:ckA 3:skA 4:ckC 5:skC
        # 6:ckNckA 7:skNskA 8:ckNskA 9:skNckA
        # 10:nskNckA 11:nckNskA 12:nckN 13:nckA 14:nckC 15:nskA
        # 16:RNckN 17:RNskN 18:RCA 19:RC
        def vc(i):
            return vecs[0:1, i:i + 1]

        for t in range(3):
            V.tensor_copy(vc(2 * t), ck3[0:1, t:t + 1])
            V.tensor_copy(vc(2 * t + 1), sk3[0:1, t:t + 1])
        V.tensor_mul(vc(6), vc(0), vc(2))
        V.tensor_mul(vc(7), vc(1), vc(3))
        V.tensor_mul(vc(8), vc(0), vc(3))
        V.tensor_mul(vc(9), vc(1), vc(2))
        V.tensor_scalar_mul(vc(10), vc(9), -1.0)
        V.tensor_scalar_mul(vc(11), vc(8), -1.0)
        V.tensor_scalar_mul(vc(12), vc(0), -1.0)
        V.tensor_scalar_mul(vc(13), vc(2), -1.0)
        V.tensor_scalar_mul(vc(14), vc(4), -1.0)
        V.tensor_scalar_mul(vc(15), vc(3), -1.0)
        V.tensor_mul(vc(16), Rv[0:1, 0:1], vc(0))
        V.tensor_mul(vc(17), Rv[0:1, 0:1], vc(1))
        V.tensor_copy(vc(18), Rv[0:1, 1:2])
        V.tensor_copy(vc(19), Rv[0:1, 2:3])
        G.partition_broadcast(Vb[:], vecs[:])

        S = {}
        for i, nm in enumerate(("ckN", "skN", "ckA", "skA", "ckC", "skC",
                                "ckNckA", "skNskA", "ckNskA", "skNckA",
                                "nskNckA", "nckNskA", "nckN", "nckA",
                                "nckC", "nskA", "RNckN", "RNskN",
                                "RCA", "RC")):
            S[nm] = Vb[:, i:i + 1]

        # ============ Phase D: C1 = B_N @ B_CA (fp16) ============
        cN, sN = ca[:, 0:NR], sa[:, 0:NR]
        cA, sA = ca[:, NR:2 * NR], sa[:, NR:2 * NR]
        cC, sC = ca[:, 2 * NR:3 * NR], sa[:, 2 * NR:3 * NR]

        def blk(t, e, lo=0, hi=NR):
            return t[:, e * NR + lo:e * NR + hi]

        pp1 = PP[:, 0 * NR:1 * NR]
        pp2 = PP[:, 1 * NR:2 * NR]
        pp3 = PP[:, 2 * NR:3 * NR]
        pp4 = PP[:, 3 * NR:4 * NR]
        V.tensor_mul(pp1, cN, cA)
        V.tensor_mul(pp2, sN, sA)
        G.tensor_mul(pp3, cN, sA)
        G.tensor_mul(pp4, sN, cA)
        c1 = C1[:]
        TS(blk(c1, 0), cA, S["skNskA"], S["ckNckA"],
           op0=OP.mult, op1=OP.add)                       # C1_00
        TS(blk(c1, 1), cA, S["nskNckA"], S["ckNskA"],
           op0=OP.mult, op1=OP.add)                       # C1_01
        V.tensor_scalar_mul(blk(c1, 2), sA, S["skN"])     # C1_02
        x = blk(c1, 3)                                    # C1_10
        A.mul(x, cN, S["skNckA"])
        STT(x, pp1, S["nckNskA"], x, op0=OP.mult, op1=OP.add)
        STT(x, pp2, S["skA"], x, op0=OP.mult, op1=OP.add)
        x = blk(c1, 4)                                    # C1_11
        A.mul(x, cN, S["skNskA"])
        STT(x, pp1, S["ckNckA"], x, op0=OP.mult, op1=OP.add)
        STT(x, pp2, S["nckA"], x, op0=OP.mult, op1=OP.add)
        STT(blk(c1, 5), pp3, S["nckN"], pp4,
            op0=OP.mult, op1=OP.subtract)                 # C1_12
        x = blk(c1, 6)                                    # C1_20
        A.mul(x, sN, S["skNckA"])
        STT(x, pp4, S["nckNskA"], x, op0=OP.mult, op1=OP.add)
        STT(x, pp3, S["nskA"], x, op0=OP.mult, op1=OP.add)
        x = blk(c1, 7)                                    # C1_21
        A.mul(x, sN, S["skNskA"])
        STT(x, pp4, S["ckNckA"], x, op0=OP.mult, op1=OP.add)
        STT(x, pp3, S["ckA"], x, op0=OP.mult, op1=OP.add)
        STT(blk(c1, 8), pp2, S["nckN"], pp1,
            op0=OP.mult, op1=OP.add)                      # C1_22

        # residue-0 of q=0 rows: C1 := B_CA(0) (alpha_CA(0)=0)
        r0s = slice(0, BPC)
        o1 = ones16[r0s, 0:1]
        V.tensor_scalar_mul(c1[r0s, 0 * NR:0 * NR + 1], o1, S["ckA"][r0s])
        V.tensor_scalar_mul(c1[r0s, 1 * NR:1 * NR + 1], o1, S["skA"][r0s])
        V.memset(c1[r0s, 2 * NR:2 * NR + 1], 0.0)
        V.tensor_scalar_mul(c1[r0s, 3 * NR:3 * NR + 1], o1, S["skA"][r0s])
        V.tensor_scalar_mul(c1[r0s, 4 * NR:4 * NR + 1], o1, S["nckA"][r0s])
        V.memset(c1[r0s, 5 * NR:5 * NR + 1], 0.0)
        V.memset(c1[r0s, 6 * NR:6 * NR + 1], 0.0)
        V.memset(c1[r0s, 7 * NR:7 * NR + 1], 0.0)
        V.memset(c1[r0s, 8 * NR:8 * NR + 1], -1.0)

        # ============ Phase E: Rres = C1 @ B_C -> RA (fp16), vm ========
        ra = RA[:]
        for i in range(3):
            (V if i != 2 else G).tensor_mul(blk(QQ[:], i),
                                            blk(c1, 3 * i + 1), cC)
            (G if i != 2 else V).tensor_mul(blk(QQ[:], 3 + i),
                                            blk(c1, 3 * i + 2), sC)
        for i in range(3):
            q1i = blk(QQ[:], i)
            q2i = blk(QQ[:], 3 + i)
            x = blk(ra, 3 * i + 0)
            A.mul(x, blk(c1, 3 * i + 0), S["ckC"])
            STT(x, q1i, S["skC"], x, op0=OP.mult, op1=OP.add)
            STT(x, q2i, S["skC"], x, op0=OP.mult, op1=OP.add)
            x = blk(ra, 3 * i + 1)
            A.mul(x, blk(c1, 3 * i + 0), S["skC"])
            STT(x, q1i, S["nckC"], x, op0=OP.mult, op1=OP.add)
            STT(x, q2i, S["nckC"], x, op0=OP.mult, op1=OP.add)
            t1 = tsq[:, i * NR:(i + 1) * NR]
            t2 = ts2[:, i * NR:(i + 1) * NR]
            V.tensor_mul(t1, blk(c1, 3 * i + 1), sC)
            G.tensor_mul(t2, blk(c1, 3 * i + 2), cC)
            V.tensor_sub(blk(ra, 3 * i + 2), t1, t2)

        # v-vectors (fp16): vm plane p = 3*vec + coord
        vmv = vm[:]
        V.tensor_scalar_mul(blk(vmv, 0), onesr[:], S["RNckN"])
        A.mul(blk(vmv, 1), cN, S["RNskN"])
        A.mul(blk(vmv, 2), sN, S["RNskN"])
        for i in range(3):
            A.mul(blk(vmv, 3 + i), blk(c1, 3 * i + 0), S["RCA"])
            A.mul(blk(vmv, 6 + i), blk(ra, 3 * i + 0), S["RC"])

        # ============ setup: selectors + mask (mid-stream) ========
        G.iota(rowid_i[:], [[0, 1]], channel_multiplier=1)
        V.tensor_copy(rowid[:], rowid_i[:])
        # colw128[p, i] = i % 32
        G.iota(colw128[:], [[0, QN], [1, BPC]], channel_multiplier=0)
        # selbq block q: (p == 32q + i%32)  <=>  (i%32 == p - 32q)
        for q in range(QN):
            TS(rowq[:], rowid[:], float(q * BPC), None, op0=OP.subtract)
            TS(selbq[:, q * 128:(q + 1) * 128], colw128[:], rowq[:, 0:1],
               None, op0=OP.is_equal)
            GTS(selbqF[:, q * 128:(q + 1) * 128], colw128[:], rowq[:, 0:1],
                None, op0=OP.is_equal)
        # selbT[b, i] = (i % 32 == b)
        TS(selbT[0:BPC, :], colw128[0:BPC, :], rowid[0:BPC, 0:1], None,
           op0=OP.is_equal)

        # mask (early): Lbc broadcast via PE, thr, maskp
        V.tensor_copy(Lf[:], Lsb[:])
        nc.tensor.matmul(PSL[:, 0:1], selbT[0:BPC, :], Lf[:, 0:1],
                         start=True, stop=True)
        G.iota(jplane_i[:], [[1, W]], channel_multiplier=0)
        V.tensor_copy(jplane[:], jplane_i[:])
        for q in range(QN):
            TS(thr[q * BPC:(q + 1) * BPC, 0:1],
               PSL[q * BPC:(q + 1) * BPC, 0:1],
               3.0, float(q * W), op0=OP.mult, op1=OP.subtract)
        TS(maskp[:], jplane[:], thr[:, 0:1], None, op0=OP.is_lt)

        G.memset(L4[:], 0.0)

        # ============ Phase F: scan machinery (fp16) ============
        # generic fused 3-term matmul group, col-split DVE/Pool
        def fused(Lf_, Rf_, Of_, Tf_, n, split=0.85, ta=None, tb=None,
                  tcx=None, eng=None):
            ta = T9a if ta is None else ta
            tb = T9b if tb is None else tb
            tcx = (T9c if ta is T9a else T9cf) if tcx is None else tcx
            if eng is not None:
                segs = [(eng, 0, n)]
            else:
                cut = min(n, max(0, int(n * split)))
                segs = []
                if cut > 0:
                    segs.append((V, 0, cut))
                if cut < n:
                    segs.append((G, cut, n))
            for E, c0, c1_ in segs:
                E.tensor_mul(Tf_(ta, c0, c1_), Lf_(0, c0, c1_),
                             Rf_(0, c0, c1_))
                E.tensor_mul(Tf_(tb, c0, c1_), Lf_(1, c0, c1_),
                             Rf_(1, c0, c1_))
                E.tensor_mul(Tf_(tcx, c0, c1_), Lf_(2, c0, c1_),
                             Rf_(2, c0, c1_))
                E.tensor_add(Of_(c0, c1_), Tf_(ta, c0, c1_),
                             Tf_(tb, c0, c1_))
                E.tensor_add(Of_(c0, c1_), Of_(c0, c1_), Tf_(tcx, c0, c1_))

        # fp16 shadow of Rres for the expansion products (Pool: feeds Wodd)
        G.tensor_copy(RA16[:], RA[:])

        # --- pair: P2[b, j] = RA[blk 2b, j] @ RA[blk 2b+1, j]
        for b in range(2):
            base = 64 * b

            def PL(k, c0, c1_, base=base):
                return _ap(ra, k * NR + base + c0,
                           [[3 * NR, 3], [0, 3], [1, c1_ - c0]])

            def PR(k, c0, c1_, base=base):
                return _ap(ra, 3 * k * NR + base + NB + c0,
                           [[0, 3], [NR, 3], [1, c1_ - c0]])

            def PO(c0, c1_, base=32 * b):
                return _ap(P2[:], base + c0,
                           [[192, 3], [64, 3], [1, c1_ - c0]])

            def PT(t, c0, c1_, base=288 * b):
                return _ap(t[:], base + c0, [[96, 3], [32, 3], [1, c1_ - c0]])

            fused(PL, PR, PO, PT, NB, ta=T9af, tb=T9bf, eng=V)

        # --- Wodd emitters (interleaved between HS steps below)
        def emit_wodd(b):
            dst, dstride = ((US, 128), (WS, 64))[b]
            base = 64 * b

            def WL(k, c0, c1_, base=base):
                return _ap(RA16[:], k * NR + base + c0,
                           [[3 * NR, 3], [0, 3], [1, c1_ - c0]])

            def WR(k, c0, c1_, base=base):
                return _ap(vmv, k * NR + base + NB + c0,
                           [[0, 3], [3 * NR, 3], [1, c1_ - c0]])

            def WO(c0, c1_, dst=dst, ds=dstride):
                return _ap(dst[:], NB + c0,
                           [[ds, 3], [3 * ds, 3], [1, c1_ - c0]])

            def WT(t, c0, c1_, base=576 + 288 * b):
                return _ap(t[:], base + c0,
                           [[32, 3], [96, 3], [1, c1_ - c0]])

            fused(WL, WR, WO, WT, NB, eng=G)

        # --- quad: P4[j] = P2[b0, j] @ P2[b1, j]
        def QL(k, c0, c1_):
            return _ap(P2[:], k * 64 + c0, [[192, 3], [0, 3], [1, c1_ - c0]])

        def QR(k, c0, c1_):
            return _ap(P2[:], 3 * k * 64 + NB + c0,
                       [[0, 3], [64, 3], [1, c1_ - c0]])

        def QO(c0, c1_):
            return _ap(P4A[:], c0, [[96, 3], [32, 3], [1, c1_ - c0]])

        def QT(t, c0, c1_):
            return _ap(t[:], c0, [[96, 3], [32, 3], [1, c1_ - c0]])

        fused(QL, QR, QO, QT, NB, eng=V)

        # --- W2 merged: out US cols 64:128 = P2even @ WS
        def emit_w2():
            A.copy(_ap(P2e2[:], 0, [[64, 9], [32, 2], [1, 32]]),
                   _ap(P2[:], 0, [[64, 9], [0, 2], [1, 32]]))
            A.copy(_ap(WS[:], 0, [[64, 9], [1, 32]]),
                   _ap(vmv, 64, [[NR, 9], [1, 32]]))
            A.copy(_ap(US[:], 0, [[128, 9], [1, 32]]),
                   _ap(vmv, 0, [[NR, 9], [1, 32]]))

            def W2L(k, c0, c1_):
                return _ap(P2e2[:], k * 64 + c0,
                           [[192, 3], [0, 3], [1, c1_ - c0]])

            def W2R(k, c0, c1_):
                return _ap(WS[:], k * 64 + c0,
                           [[0, 3], [192, 3], [1, c1_ - c0]])

            def W2O(c0, c1_):
                return _ap(US[:], 64 + c0,
                           [[128, 3], [384, 3], [1, c1_ - c0]])

            def W2T(t, c0, c1_):
                return _ap(t[:], 288 + c0,
                           [[64, 3], [192, 3], [1, c1_ - c0]])

            fused(W2L, W2R, W2O, W2T, 2 * NB, eng=V)

        # --- Hillis-Steele over 32 quads, expansions interleaved
        def hs_step(srcb, dstb, s):
            n = NB - s
            sv = srcb.rearrange("p (e j) -> p e j", e=9)
            dv = dstb.rearrange("p (e j) -> p e j", e=9)
            V.tensor_copy(dv[:, :, 0:s], sv[:, :, 0:s])

            def HL(k, c0, c1_):
                return _ap(srcb, k * NB + c0,
                           [[96, 3], [0, 3], [1, c1_ - c0]])

            def HR(k, c0, c1_):
                return _ap(srcb, 3 * k * NB + s + c0,
                           [[0, 3], [32, 3], [1, c1_ - c0]])

            def HO(c0, c1_):
                return _ap(dstb, s + c0, [[96, 3], [32, 3], [1, c1_ - c0]])

            def HT(t, c0, c1_):
                return _ap(t[:], c0, [[96, 3], [32, 3], [1, c1_ - c0]])

            fused(HL, HR, HO, HT, n, eng=V)

        bufs = [P4A, P4B]
        emit_wodd(0)
        emit_wodd(1)
        hs_step(bufs[0][:], bufs[1][:], 1)
        hs_step(bufs[1][:], bufs[0][:], 2)
        hs_step(bufs[0][:], bufs[1][:], 4)
        hs_step(bufs[1][:], bufs[0][:], 8)
        hs_step(bufs[0][:], bufs[1][:], 16)
        emit_w2()
        Rscan = bufs[1][:]    # P4B

        # --- rotation fixup: E gathered to ALL rows, F chain, slice Fbc ---
        for q in range(QN):
            nc.tensor.matmul(
                PSg[:, q * 9:(q + 1) * 9],
                selbq[:, q * 128:(q + 1) * 128],
                _ap(Rscan, NB - 1, [[NB, 9]]), start=True, stop=True)
        V.tensor_copy(Estack[:, 0:36], PSg[:, 0:36])
        V.tensor_copy(Fstack[:, 0:9], Estack[:, 0:9])
        fs = Fstack[:]
        es = Estack[:]

        def ap2(base_ap, off, dims):
            return _ap(base_ap, off, dims)

        for q in (1, 2):
            FL = lambda k: ap2(fs, (q - 1) * 9 + k, [[3, 3], [0, 3]])
            ER = lambda k: ap2(es, q * 9 + 3 * k, [[0, 3], [1, 3]])
            MT = lambda t: ap2(t[:], 0, [[3, 3], [1, 3]])
            FO = ap2(fs, q * 9, [[3, 3], [1, 3]])
            V.tensor_mul(MT(mt0), FL(0), ER(0))
            V.tensor_mul(MT(mt1), FL(1), ER(1))
            V.tensor_add(MT(mt0), MT(mt0), MT(mt1))
            V.tensor_mul(MT(mt1), FL(2), ER(2))
            V.tensor_add(FO, MT(mt0), MT(mt1))
        # Fbc: rows 0:32 identity; quarter q rows take F_q slice
        V.memset(Fbc[0:BPC, 0:9], 0.0)
        V.memset(bass.AP(Fbc[:].tensor, Fbc[:].offset,
                         [[Fbc[:].ap[0][0], BPC], [4, 3]]), 1.0)
        for q in (1, 2, 3):
            V.tensor_copy(Fbc[q * BPC:(q + 1) * BPC, 0:9],
                          Fstack[q * BPC:(q + 1) * BPC,
                                 (q - 1) * 9:q * 9])

        # --- L4: shifted replicate of Rscan ---
        A.copy(_ap(L4[:], 1, [[128, 9], [32, 4], [1, NB - 1]]),
               _ap(Rscan, 0, [[NB, 9], [0, 4], [1, NB - 1]]))

        # --- U merged: Uloc[c][j*128 + g*32 + r] = L4 @ US
        def UL(k, c0, c1_):
            return _ap(L4[:], k * 128 + c0,
                       [[384, 3], [0, 3], [1, c1_ - c0]])

        def UR(k, c0, c1_):
            return _ap(US[:], k * 128 + c0,
                       [[0, 3], [384, 3], [1, c1_ - c0]])

        def UO(c0, c1_):
            return _ap(Uloc[:], c0, [[384, 3], [128, 3], [1, c1_ - c0]])

        def UT(t, c0, c1_):
            return _ap(t[:], c0, [[384, 3], [128, 3], [1, c1_ - c0]])

        fused(UL, UR, UO, UT, 4 * NB, split=0.9)

        # seeds: r=0 of each g-block: identity prefix -> u = US col g*32
        A.copy(_ap(Uloc[:], 0, [[384, 3], [128, 3], [32, 4]]),
               _ap(US[:], 0, [[128, 3], [384, 3], [32, 4]]))
        # atom 0 of whole chain (q=0 rows) at origin
        V.memset(bass.AP(Uloc[:].tensor, Uloc[:].offset,
                         [[Uloc[:].ap[0][0], BPC], [384, 3]]), 0.0)

        # ============ Phase G: tail ============
        # F-apply fused with blocked->ordered reorder:
        #   Uord_c[a] = F_c0*Ux[b(a)] + F_c1*Uy[b(a)] + F_c2*Uz[b(a)]
        # (walk order (r,g,j): out stride-1-ish ordered, ins blocked)
        def ordw(t, c):
            return _ap(t[:], c * W, [[12, 32], [3, 4], [1, 3]])

        def blkw(t, c):
            return _ap(t[:], c * W, [[1, 32], [32, 4], [128, 3]])

        for c in range(3):
            x = T9a[:, c * W:(c + 1) * W]    # fp16 scratch (blocked)
            y = T9b[:, c * W:(c + 1) * W]
            z = T9c[:, c * W:(c + 1) * W]
            A.mul(x, Uloc[:, 0:W], Fbc[:, 3 * c + 0:3 * c + 1])
            TS(y, Uloc[:, W:2 * W], Fbc[:, 3 * c + 1:3 * c + 2], None,
               op0=OP.mult)
            GTS(z, Uloc[:, 2 * W:3 * W], Fbc[:, 3 * c + 2:3 * c + 3], None,
                op0=OP.mult)
            V.tensor_add(x, x, y)
            RS = 20
            V.tensor_add(_ap(Uord[:], c * W, [[12, RS], [3, 4], [1, 3]]),
                         _ap(x, 0, [[1, RS], [32, 4], [128, 3]]),
                         _ap(z, 0, [[1, RS], [32, 4], [128, 3]]))
            G.tensor_add(
                _ap(Uord[:], c * W + 12 * RS, [[12, 32 - RS], [3, 4], [1, 3]]),
                _ap(x, RS, [[1, 32 - RS], [32, 4], [128, 3]]),
                _ap(z, RS, [[1, 32 - RS], [32, 4], [128, 3]]))

        # quarter sums of rotated u -> Pinc via gather + cumsum
        for c in range(3):
            V.tensor_reduce(Sall[:, c:c + 1], Uord[:, c * W:(c + 1) * W],
                            axis=mybir.AxisListType.X, op=OP.add)
        A.copy(Sall16[:], Sall[:])
        for q in range(QN):
            nc.tensor.matmul(
                PSp[:, q * 3:(q + 1) * 3],
                selbq[:, q * 128:(q + 1) * 128],
                Sall16[:, 0:3], start=True, stop=True)
        V.tensor_copy(Sg[:, 0:9], PSp[:, 0:9])
        # cumulative sums: cumst block q-1 = Pinc_{q}
        V.tensor_copy(cumst[:, 0:3], Sg[:, 0:3])
        V.tensor_add(cumst[:, 3:6], cumst[:, 0:3], Sg[:, 3:6])
        V.tensor_add(cumst[:, 6:9], cumst[:, 3:6], Sg[:, 6:9])
        # Pincb: rows 0:32 zero; quarter q rows take Pinc_q slice
        V.memset(FtPb[0:BPC, 0:3], 0.0)
        for q in (1, 2, 3):
            V.tensor_copy(FtPb[q * BPC:(q + 1) * BPC, 0:3],
                          cumst[q * BPC:(q + 1) * BPC, (q - 1) * 3:q * 3])

        # masked scan seeded with Pinc -> final coords; store
        for c in range(3):
            V.tensor_tensor_scan(
                Pall[:, c * W:(c + 1) * W], Uord[:, c * W:(c + 1) * W],
                maskp[:], FtPb[:, c:c + 1], op0=OP.add, op1=OP.mult)
            (nc.sync if c != 1 else nc.scalar).dma_start(
                out[:, c * W:(c + 1) * W], Pall[:, c * W:(c + 1) * W])

def _prep_alpha(input):
    # alphaN[r]=psi[r-1], alphaCA[r]=omega[r-1] (0 at r=0), alphaC[r]=phi[r];
    # then mod-4 block permutation within each 128-residue quarter.
    phi, psi, om = input[:, 0], input[:, 1], input[:, 2]
    z1 = np.zeros((input.shape[0], 1), np.float32)
    aN = np.concatenate([z1, psi[:, :-1]], axis=1)
    aCA = np.concatenate([z1, om[:, :-1]], axis=1)
    alpha = np.stack([aN, aCA, phi], axis=1)          # [B, 3, 512]
    alpha = alpha.reshape(-1, 3, QN, NR)
    perm = np.arange(NR).reshape(NB, 4).T.reshape(-1)  # sigma^-1: col->r
    alpha = alpha[..., perm]                           # blocked columns
    return alpha.transpose(0, 2, 1, 3)                 # [B, QN, 3, NR]


def _shard_alpha(alpha, i):
    sl = slice(i * BPC, (i + 1) * BPC)
    return np.ascontiguousarray(
        alpha[sl].transpose(1, 0, 2, 3).reshape(QN * BPC, 3 * NR))


def _get_nc():
    if "nc" not in _CACHE:
        _CACHE["nc"] = _build_graph()
    return _CACHE["nc"]


def kernel(input, param, angles_length, trace=False):
    input = np.ascontiguousarray(input, dtype=np.float32)
    param = np.ascontiguousarray(param, dtype=np.float32)
    angles_length = np.ascontiguousarray(angles_length, dtype=np.int32)
    nc = _get_nc()
    alpha = _prep_alpha(input)
    in_maps = []
    for i in range(NCORES):
        sl = slice(i * BPC, (i + 1) * BPC)
        in_maps.append({
            "input": _shard_alpha(alpha, i),
            "param": param,
            "angles_length": angles_length[sl],
        })
    res = run_bass_kernel_spmd(nc, in_maps, core_ids=list(range(NCORES)),
                               trace=trace)
    outs = []
    for i in range(NCORES):
        r = res.results[i]["out"]          # [(q,b), (c,j)]
        r = r.reshape(QN, BPC, 3, W)
        r = np.transpose(r, (1, 0, 3, 2)).reshape(BPC, 3 * QN * W)
        outs.append(r)
    full = np.concatenate(outs, axis=0).astype(np.float32)
    if trace:
        kernel._last_exec_ns = res.exec_time_ns
    return full


kernel._last_exec_ns = None


# revision 37
# speedup vs baseline: 1.2003x; 1.0984x over previous
"""Trainium2 Bass kernel for Angles2Backbone (NeRF chain forward).

Full inputs: input [256,3,512] f32, param [6] f32, angles_length [256] i32.
Output: [256, 4608] f32  (coords of 1536 backbone atoms x 3, masked).

Sharding: pure data parallel over batch - 32 proteins per core x 8 cores.

Per-core algorithm (v3: fp16 scan machinery, mod-4 blocked residue layout):
  - 128 partitions = (quarter q)*32 + protein b; each row owns 128 residues.
  - Residue r of a quarter lives at column sigma(r) = 32*(r%4) + r//4
    (host-side permutation), so every pair/quad/expansion op reads and
    writes stride-1 column blocks -> DVE 2x fp16 mode throughout.
  - Trig via sin LUT at a/8, a/4 + double-angle chains (fp16).
  - Pre-pass builds per-residue rotation Rres (fp16) from scalar-folded
    bilinear terms; v-vectors (per-atom translations) in fp16.
  - pair (mod-4 blocks) -> P2, quad -> P4, Hillis-Steele over 32 quads.
  - Vector expansions Wodd/W2 written straight into a fused source tensor
    US so the superquad expansion is 5 big ops against a replicated,
    shifted prefix tensor L4.
  - Positions: blocked u -> reorder copy -> masked tensor_tensor_scan
    seeded with F^T.Pinc (cross-quarter fixup folded into the scan),
    then frame rotation F and store.
  - Cross-partition moves via PE matmuls only (no SBUF-SBUF DMA).
"""

import sys

sys.path.insert(0, "/opt/trn_rl_repo")

import numpy as np
import concourse.bass as bass
import concourse.bacc as bacc
import concourse.mybir as mybir
from concourse import tile
from concourse.bass_utils import run_bass_kernel_spmd

F32 = mybir.dt.float32
F16 = mybir.dt.float16
I32 = mybir.dt.int32
AF = mybir.ActivationFunctionType
OP = mybir.AluOpType

NCORES = 8
BPC = 32          # proteins per core
L = 512           # residues per protein
QN = 4            # chain quarters per protein (partition groups)
W = 384           # atoms per quarter
NR = 128          # residues per quarter
NB = 32           # columns per mod-4 block
PI = float(np.pi)

_CACHE = {}


def _build_graph():
    nc = bacc.Bacc("TRN2", target_bir_lowering=False, debug=False,
                   num_devices=NCORES)
    inp = nc.dram_tensor("input", [QN * BPC, 3 * NR], F32,
                         kind="ExternalInput").ap()
    par = nc.dram_tensor("param", [6], F32, kind="ExternalInput").ap()
    alen = nc.dram_tensor("angles_length", [BPC], I32,
                          kind="ExternalInput").ap()
    out = nc.dram_tensor("out", [QN * BPC, 3 * W], F32,
                         kind="ExternalOutput").ap()
    with tile.TileContext(nc) as tc:
        _emit(nc, tc, inp, par, alen, out)
    nc.compile()
    return nc


def _ap(base_ap, off, dims):
    return bass.AP(base_ap.tensor, base_ap.offset + off,
                   [list(base_ap.ap[0])] + [list(d) for d in dims])


def _emit(nc, tc, inp, par, alen, out):
    import contextlib
    ctx = contextlib.ExitStack()
    with ctx:
        main = ctx.enter_context(tc.tile_pool(name="main", bufs=1))
        psum = ctx.enter_context(tc.tile_pool(name="psum", bufs=1,
                                              space="PSUM"))

        # ---------------- tiles ----------------
        alpha = main.tile([128, 3 * NR], F32, tag="alpha")
        ca = main.tile([128, 3 * NR], F32, tag="ca")
        sa = main.tile([128, 3 * NR], F32, tag="sa")
        # trig scratch (per type block)
        ts8 = main.tile([128, 3 * NR], F32, tag="ts8")   # s8 then sq8
        ts4 = main.tile([128, 3 * NR], F32, tag="ts4")   # s4 then s4*2
        tsq = main.tile([128, 3 * NR], F32, tag="tsq")   # squares
        ts2 = main.tile([128, 3 * NR], F32, tag="ts2")   # s2 then s2*2

        PP = main.tile([128, 4 * NR], F32, tag="PP")
        C1 = main.tile([128, 9 * NR], F32, tag="C1")
        QQ = main.tile([128, 6 * NR], F32, tag="QQ")
        RA = main.tile([128, 9 * NR], F32, tag="RA")     # Rres f32
        vm = main.tile([128, 9 * NR], F16, tag="vm")
        P2 = main.tile([128, 9 * 2 * NB], F16, tag="P2")
        P2e2 = main.tile([128, 9 * 2 * NB], F16, tag="P2e2")
        P4A = main.tile([128, 9 * NB], F16, tag="P4A")
        P4B = main.tile([128, 9 * NB], F16, tag="P4B")
        US = main.tile([128, 9 * 4 * NB], F16, tag="US")
        WS = main.tile([128, 9 * 2 * NB], F16, tag="WS")
        L4 = main.tile([128, 9 * 4 * NB], F16, tag="L4")
        T9a = main.tile([128, 9 * 4 * NB], F16, tag="T9a")
        T9b = main.tile([128, 9 * 4 * NB], F16, tag="T9b")
        T9c = main.tile([128, 9 * 4 * NB], F16, tag="T9c")
        T9af = main.tile([128, 18 * NB], F32, tag="T9af")
        T9bf = main.tile([128, 18 * NB], F32, tag="T9bf")
        T9cf = main.tile([128, 18 * NB], F32, tag="T9cf")
        Uloc = main.tile([128, 3 * W], F16, tag="Uloc")  # blocked u
        Uord = main.tile([128, 3 * W], F16, tag="Uord")  # ordered u
        Pall = main.tile([128, 3 * W], F32, tag="Pall")  # scanned (masked)
        Pmall = main.tile([128, 3 * W], F32, tag="Pmall")

        jplane_i = main.tile([128, W], I32, tag="jplane_i")
        jplane = main.tile([128, W], F32, tag="jplane")
        maskp = main.tile([128, W], F16, tag="maskp")
        thr = main.tile([128, 1], F32, tag="thr")
        Lsb = main.tile([BPC, 1], I32, tag="Lsb")
        Lf = main.tile([BPC, 1], F32, tag="Lf")
        Psb = main.tile([1, 6], F32, tag="Psb")
        kv = main.tile([1, 3], F32, tag="kv")
        Rv = main.tile([1, 3], F32, tag="Rv")
        NSC = 24
        vecs = main.tile([1, NSC], F32, tag="vecs")
        Vb = main.tile([128, NSC], F32, tag="Vb")
        zb1 = main.tile([1, 1], F32, tag="zb1")
        zb128 = main.tile([128, 1], F32, tag="zb128")
        warm = main.tile([1, 1], F32, tag="warm")
        ones16 = main.tile([128, 1], F16, tag="ones16")
        onesr = main.tile([128, NR], F16, tag="onesr")

        # selectors
        rowid_i = main.tile([128, 1], I32, tag="rowid_i")
        rowid = main.tile([128, 1], F32, tag="rowid")
        rowq = main.tile([128, 1], F32, tag="rowq")
        colw128 = main.tile([128, 128], I32, tag="colw128")   # i % 32
        selbq = main.tile([128, QN * 128], F16, tag="selbq")  # all-row gather
        selbqF = main.tile([128, QN * 128], F32, tag="selbqF")
        selbT = main.tile([BPC, 128], F32, tag="selbT")       # Lbc bcast

        # cross-quarter fixup (redundantly on all 128 rows, f32)
        Estack = main.tile([128, 36], F32, tag="Estack")
        Fstack = main.tile([128, 27], F32, tag="Fstack")
        Fbc = main.tile([128, 9], F32, tag="Fbc")
        Sg = main.tile([128, 12], F32, tag="Sg")
        Gv = main.tile([128, 9], F32, tag="Gv")
        cumst = main.tile([128, 9], F32, tag="cumst")
        FtP = main.tile([128, 9], F32, tag="FtP")
        FtPb = main.tile([128, 3], F32, tag="FtPb")
        Sall = main.tile([128, 3], F32, tag="Sall")
        Sall16 = main.tile([128, 3], F16, tag="Sall16")
        mt0 = main.tile([128, 9], F32, tag="mt0")
        mt1 = main.tile([128, 9], F32, tag="mt1")

        PSg = psum.tile([128, 36], F32, tag="PSg")
        PSp = psum.tile([128, 12], F32, tag="PSp")
        PSL = psum.tile([128, 1], F32, tag="PSL")

        V = nc.vector
        G = nc.gpsimd
        A = nc.scalar
        STT = nc.vector.scalar_tensor_tensor
        TS = nc.vector.tensor_scalar
        GTS = nc.gpsimd.tensor_scalar

        # ============ Phase A: DMAs + ACT warmup + setup ============
        nc.sync.dma_start(alpha[:], inp[:])
        nc.sync.dma_start(Psb[:], par[:])
        nc.sync.dma_start(Lsb[:], alen[:])
        V.memset(zb1[:], 0.0)
        V.memset(zb128[:], 0.0)
        # trigger the Sin table load immediately (Copy set loads after sins)
        A.activation(warm[:], zb1[:], AF.Sin, bias=zb1[:])

        G.memset(ones16[:], 1.0)
        G.memset(onesr[:], 1.0)
        G.iota(rowid_i[:], [[0, 1]], channel_multiplier=1)
        G.iota(colw128[:], [[0, QN], [1, BPC]], channel_multiplier=0)
        V.tensor_copy(rowid[:], rowid_i[:])

        # ============ Phase B: trig (fp16 double-angle chains) ========
        # engines per type block: ACT does sin(a/8), sin(a/4), sq8;
        # chains on DVE (N, A) and Pool (C).
        def trig_sins(t):
            bs = slice(t * NR, (t + 1) * NR)
            avb = alpha[:, bs]
            A.activation(ts8[:, bs], avb, AF.Sin, bias=zb128[:], scale=0.125)
            A.activation(ts4[:, bs], avb, AF.Sin, bias=zb128[:], scale=0.25)

        def trig_chain(t, E):
            TSx = V.tensor_scalar if E is V else G.tensor_scalar
            bs = slice(t * NR, (t + 1) * NR)
            s8, s4, sq, s2 = ts8[:, bs], ts4[:, bs], tsq[:, bs], ts2[:, bs]
            cab, sab = ca[:, bs], sa[:, bs]
            E.tensor_mul(sq, s8, s8)                           # sin^2(a/8)
            TSx(cab, sq, -2.0, 1.0, op0=OP.mult, op1=OP.add)   # c4
            TSx(s4, s4, 2.0, None, op0=OP.mult)                # 2*s4
            E.tensor_mul(s2, s4, cab)                          # s2 = 2 s4 c4
            E.tensor_mul(sq, s4, s4)                           # (2 s4)^2
            TSx(cab, sq, -0.5, 1.0, op0=OP.mult, op1=OP.add)   # c2
            TSx(s2, s2, 2.0, None, op0=OP.mult)                # 2*s2
            E.tensor_mul(sab, s2, cab)                         # s1
            E.tensor_mul(sq, s2, s2)                           # (2 s2)^2
            TSx(cab, sq, -0.5, 1.0, op0=OP.mult, op1=OP.add)   # c1

        for t in range(3):
            trig_sins(t)
        trig_chain(0, V)
        trig_chain(1, V)

        # ============ Phase C: param scalars ============
        for t, idx in enumerate((5, 1, 3)):   # kappa: CA_C_N, C_N_CA, N_CA_C
            V.tensor_copy(kv[0:1, t:t + 1], Psb[0:1, idx:idx + 1])
        for t, idx in enumerate((4, 0, 2)):   # R: R_C_N, R_N_CA, R_CA_C
            V.tensor_copy(Rv[0:1, t:t + 1], Psb[0:1, idx:idx + 1])
        sk3 = main.tile([1, 3], F32, tag="sk3")
        ck3 = main.tile([1, 3], F32, tag="ck3")
        kvr = main.tile([1, 3], F32, tag="kvr")
        A.activation(sk3[:], kv[0:1, 0:3], AF.Sin, bias=zb1[:])
        A.activation(kvr[:], kv[0:1, 0:3], AF.Sin, bias=zb1[:], scale=0.5)
        A.copy(warm[:], zb1[:])     # Copy-set LUT load, off critical path
        V.tensor_mul(kvr[:], kvr[:], kvr[:])
        V.tensor_scalar(ck3[:], kvr[:], -2.0, 1.0, op0=OP.mult, op1=OP.add)

        # scalar slots in vecs[1, NSC]:
        # 0:ckN 1:skN 2:ckA 3:skA 4:ckC 5:skC
        # 6:ckNckA 7:ckNskA 8:skNckA 9:skNskA
        # 10:nckN 11:nckA 12:nckC 13:nskA 14:nckNskA 15:nskNckA
        # 16:RNckN 17:RNskN 18:RCA 19:RC
        def vc(i):
            return vecs[0:1, i:i + 1]

        # interleave ck/sk into slots 0..5
        V.tensor_copy(_ap(vecs[:], 0, [[2, 3]]), ck3[0:1, 0:3])
        V.tensor_copy(_ap(vecs[:], 1, [[2, 3]]), sk3[0:1, 0:3])
        # outer product (ckN,skN) x (ckA,skA) -> slots 6..9
        V.tensor_mul(_ap(vecs[:], 6, [[2, 2], [1, 2]]),
                     _ap(vecs[:], 0, [[1, 2], [0, 2]]),
                     _ap(vecs[:], 2, [[0, 2], [1, 2]]))
        # negations: 10..12 = -(ckN,ckA,ckC) ; 13 = -skA ; 14,15 = -(7,8)
        V.tensor_scalar(_ap(vecs[:], 10, [[1, 3]]),
                        _ap(vecs[:], 0, [[2, 3]]), -1.0, None, op0=OP.mult)
        V.tensor_scalar(vc(13), vc(3), -1.0, None, op0=OP.mult)
        V.tensor_scalar(_ap(vecs[:], 14, [[1, 2]]),
                        _ap(vecs[:], 7, [[1, 2]]), -1.0, None, op0=OP.mult)
        # 16,17 = RN * (ckN, skN) ; 18,19 = RCA, RC
        V.tensor_mul(_ap(vecs[:], 16, [[1, 2]]),
                     _ap(Rv[:], 0, [[0, 2]]), _ap(vecs[:], 0, [[1, 2]]))
        V.tensor_copy(_ap(vecs[:], 18, [[1, 2]]), Rv[0:1, 1:3])
        G.partition_broadcast(Vb[:], vecs[:])
        trig_chain(2, G)

        S = {}
        for i, nm in enumerate(("ckN", "skN", "ckA", "skA", "ckC", "skC",
                                "ckNckA", "ckNskA", "skNckA", "skNskA",
                                "nckN", "nckA", "nckC", "nskA",
                                "nckNskA", "nskNckA", "RNckN", "RNskN",
                                "RCA", "RC")):
            S[nm] = Vb[:, i:i + 1]

        # ============ Phase D: C1 = B_N @ B_CA (fp16) ============
        cN, sN = ca[:, 0:NR], sa[:, 0:NR]
        cA, sA = ca[:, NR:2 * NR], sa[:, NR:2 * NR]
        cC, sC = ca[:, 2 * NR:3 * NR], sa[:, 2 * NR:3 * NR]

        def blk(t, e, lo=0, hi=NR):
            return t[:, e * NR + lo:e * NR + hi]

        pp1 = PP[:, 0 * NR:1 * NR]
        pp2 = PP[:, 1 * NR:2 * NR]
        pp3 = PP[:, 2 * NR:3 * NR]
        pp4 = PP[:, 3 * NR:4 * NR]
        V.tensor_mul(pp1, cN, cA)
        V.tensor_mul(pp2, sN, sA)
        G.tensor_mul(pp3, cN, sA)
        G.tensor_mul(pp4, sN, cA)
        c1 = C1[:]
        TS(blk(c1, 0), cA, S["skNskA"], S["ckNckA"],
           op0=OP.mult, op1=OP.add)                       # C1_00
        TS(blk(c1, 1), cA, S["nskNckA"], S["ckNskA"],
           op0=OP.mult, op1=OP.add)                       # C1_01
        V.tensor_scalar_mul(blk(c1, 2), sA, S["skN"])     # C1_02
        x = blk(c1, 3)                                    # C1_10
        A.mul(x, cN, S["skNckA"])
        STT(x, pp1, S["nckNskA"], x, op0=OP.mult, op1=OP.add)
        STT(x, pp2, S["skA"], x, op0=OP.mult, op1=OP.add)
        x = blk(c1, 4)                                    # C1_11
        A.mul(x, cN, S["skNskA"])
        STT(x, pp1, S["ckNckA"], x, op0=OP.mult, op1=OP.add)
        STT(x, pp2, S["nckA"], x, op0=OP.mult, op1=OP.add)
        STT(blk(c1, 5), pp3, S["nckN"], pp4,
            op0=OP.mult, op1=OP.subtract)                 # C1_12
        x = blk(c1, 6)                                    # C1_20
        A.mul(x, sN, S["skNckA"])
        STT(x, pp4, S["nckNskA"], x, op0=OP.mult, op1=OP.add)
        STT(x, pp3, S["nskA"], x, op0=OP.mult, op1=OP.add)
        x = blk(c1, 7)                                    # C1_21
        A.mul(x, sN, S["skNskA"])
        STT(x, pp4, S["ckNckA"], x, op0=OP.mult, op1=OP.add)
        STT(x, pp3, S["ckA"], x, op0=OP.mult, op1=OP.add)
        STT(blk(c1, 8), pp2, S["nckN"], pp1,
            op0=OP.mult, op1=OP.add)                      # C1_22

        # residue-0 of q=0 rows: C1 := B_CA(0) (alpha_CA(0)=0)
        r0s = slice(0, BPC)
        o1 = ones16[r0s, 0:1]
        V.tensor_scalar_mul(c1[r0s, 0 * NR:0 * NR + 1], o1, S["ckA"][r0s])
        V.tensor_scalar_mul(c1[r0s, 1 * NR:1 * NR + 1], o1, S["skA"][r0s])
        V.memset(c1[r0s, 2 * NR:2 * NR + 1], 0.0)
        V.tensor_scalar_mul(c1[r0s, 3 * NR:3 * NR + 1], o1, S["skA"][r0s])
        V.tensor_scalar_mul(c1[r0s, 4 * NR:4 * NR + 1], o1, S["nckA"][r0s])
        V.memset(c1[r0s, 5 * NR:5 * NR + 1], 0.0)
        V.memset(c1[r0s, 6 * NR:6 * NR + 1], 0.0)
        V.memset(c1[r0s, 7 * NR:7 * NR + 1], 0.0)
        V.memset(c1[r0s, 8 * NR:8 * NR + 1], -1.0)

        # ============ Phase E: Rres = C1 @ B_C -> RA (fp16), vm ========
        ra = RA[:]
        for i in range(3):
            (V if i != 2 else G).tensor_mul(blk(QQ[:], i),
                                            blk(c1, 3 * i + 1), cC)
            (G if i != 2 else V).tensor_mul(blk(QQ[:], 3 + i),
                                            blk(c1, 3 * i + 2), sC)
        for i in range(3):
            q1i = blk(QQ[:], i)
            q2i = blk(QQ[:], 3 + i)
            x = blk(ra, 3 * i + 0)
            A.mul(x, blk(c1, 3 * i + 0), S["ckC"])
            STT(x, q1i, S["skC"], x, op0=OP.mult, op1=OP.add)
            STT(x, q2i, S["skC"], x, op0=OP.mult, op1=OP.add)
            x = blk(ra, 3 * i + 1)
            A.mul(x, blk(c1, 3 * i + 0), S["skC"])
            STT(x, q1i, S["nckC"], x, op0=OP.mult, op1=OP.add)
            STT(x, q2i, S["nckC"], x, op0=OP.mult, op1=OP.add)
            t1 = tsq[:, i * NR:(i + 1) * NR]
            t2 = ts2[:, i * NR:(i + 1) * NR]
            V.tensor_mul(t1, blk(c1, 3 * i + 1), sC)
            G.tensor_mul(t2, blk(c1, 3 * i + 2), cC)
            V.tensor_sub(blk(ra, 3 * i + 2), t1, t2)

        # v-vectors (fp16): vm plane p = 3*vec + coord
        vmv = vm[:]
        V.tensor_scalar_mul(blk(vmv, 0), onesr[:], S["RNckN"])
        A.mul(blk(vmv, 1), cN, S["RNskN"])
        A.mul(blk(vmv, 2), sN, S["RNskN"])
        for i in range(3):
            A.mul(blk(vmv, 3 + i), blk(c1, 3 * i + 0), S["RCA"])
            A.mul(blk(vmv, 6 + i), blk(ra, 3 * i + 0), S["RC"])

        # ============ setup: selectors + mask (mid-stream) ========
        # selbq block q: (p == 32q + i%32)  <=>  (i%32 == p - 32q)
        for q in range(QN):
            TS(rowq[:], rowid[:], float(q * BPC), None, op0=OP.subtract)
            TS(selbq[:, q * 128:(q + 1) * 128], colw128[:], rowq[:, 0:1],
               None, op0=OP.is_equal)
            GTS(selbqF[:, q * 128:(q + 1) * 128], colw128[:], rowq[:, 0:1],
                None, op0=OP.is_equal)
        # selbT[b, i] = (i % 32 == b)
        TS(selbT[0:BPC, :], colw128[0:BPC, :], rowid[0:BPC, 0:1], None,
           op0=OP.is_equal)

        # mask (early): Lbc broadcast via PE, thr, maskp
        V.tensor_copy(Lf[:], Lsb[:])
        nc.tensor.matmul(PSL[:, 0:1], selbT[0:BPC, :], Lf[:, 0:1],
                         start=True, stop=True)
        G.iota(jplane_i[:], [[1, W]], channel_multiplier=0)
        V.tensor_copy(jplane[:], jplane_i[:])
        for q in range(QN):
            TS(thr[q * BPC:(q + 1) * BPC, 0:1],
               PSL[q * BPC:(q + 1) * BPC, 0:1],
               3.0, float(q * W), op0=OP.mult, op1=OP.subtract)
        TS(maskp[:], jplane[:], thr[:, 0:1], None, op0=OP.is_lt)

        G.memset(L4[:], 0.0)

        # ============ Phase F: scan machinery (fp16) ============
        # generic fused 3-term matmul group, col-split DVE/Pool
        def fused(Lf_, Rf_, Of_, Tf_, n, split=0.85, ta=None, tb=None,
                  tcx=None, eng=None):
            ta = T9a if ta is None else ta
            tb = T9b if tb is None else tb
            tcx = (T9c if ta is T9a else T9cf) if tcx is None else tcx
            if eng is not None:
                segs = [(eng, 0, n)]
            else:
                cut = min(n, max(0, int(n * split)))
                segs = []
                if cut > 0:
                    segs.append((V, 0, cut))
                if cut < n:
                    segs.append((G, cut, n))
            for E, c0, c1_ in segs:
                E.tensor_mul(Tf_(ta, c0, c1_), Lf_(0, c0, c1_),
                             Rf_(0, c0, c1_))
                E.tensor_mul(Tf_(tb, c0, c1_), Lf_(1, c0, c1_),
                             Rf_(1, c0, c1_))
                E.tensor_mul(Tf_(tcx, c0, c1_), Lf_(2, c0, c1_),
                             Rf_(2, c0, c1_))
                E.tensor_add(Of_(c0, c1_), Tf_(ta, c0, c1_),
                             Tf_(tb, c0, c1_))
                E.tensor_add(Of_(c0, c1_), Of_(c0, c1_), Tf_(tcx, c0, c1_))

        # --- pair: P2[b, j] = RA[blk 2b, j] @ RA[blk 2b+1, j]
        for b in range(2):
            base = 64 * b

            def PL(k, c0, c1_, base=base):
                return _ap(ra, k * NR + base + c0,
                           [[3 * NR, 3], [0, 3], [1, c1_ - c0]])

            def PR(k, c0, c1_, base=base):
                return _ap(ra, 3 * k * NR + base + NB + c0,
                           [[0, 3], [NR, 3], [1, c1_ - c0]])

            def PO(c0, c1_, base=32 * b):
                return _ap(P2[:], base + c0,
                           [[192, 3], [64, 3], [1, c1_ - c0]])

            def PT(t, c0, c1_, base=288 * b):
                return _ap(t[:], base + c0, [[96, 3], [32, 3], [1, c1_ - c0]])

            fused(PL, PR, PO, PT, NB, ta=T9af, tb=T9bf, eng=V)

        # --- Wodd emitters
        def emit_wodd(b, eng):
            dst, dstride = ((US, 128), (WS, 64))[b]
            base = 64 * b

            def WL(k, c0, c1_, base=base):
                return _ap(ra, k * NR + base + c0,
                           [[3 * NR, 3], [0, 3], [1, c1_ - c0]])

            def WR(k, c0, c1_, base=base):
                return _ap(vmv, k * NR + base + NB + c0,
                           [[0, 3], [3 * NR, 3], [1, c1_ - c0]])

            def WO(c0, c1_, dst=dst, ds=dstride):
                return _ap(dst[:], NB + c0,
                           [[ds, 3], [3 * ds, 3], [1, c1_ - c0]])

            def WT(t, c0, c1_, base=576 + 288 * b):
                return _ap(t[:], base + c0,
                           [[32, 3], [96, 3], [1, c1_ - c0]])

            fused(WL, WR, WO, WT, NB, eng=eng)

        # --- quad: P4[j] = P2[b0, j] @ P2[b1, j]
        def QL(k, c0, c1_):
            return _ap(P2[:], k * 64 + c0, [[192, 3], [0, 3], [1, c1_ - c0]])

        def QR(k, c0, c1_):
            return _ap(P2[:], 3 * k * 64 + NB + c0,
                       [[0, 3], [64, 3], [1, c1_ - c0]])

        def QO(c0, c1_):
            return _ap(P4A[:], c0, [[96, 3], [32, 3], [1, c1_ - c0]])

        def QT(t, c0, c1_):
            return _ap(t[:], c0, [[96, 3], [32, 3], [1, c1_ - c0]])

        fused(QL, QR, QO, QT, NB, eng=V)

        # --- W2 merged: out US cols 64:128 = P2even @ WS
        def emit_w2():
            A.copy(_ap(P2e2[:], 0, [[64, 9], [32, 2], [1, 32]]),
                   _ap(P2[:], 0, [[64, 9], [0, 2], [1, 32]]))
            A.copy(_ap(WS[:], 0, [[64, 9], [1, 32]]),
                   _ap(vmv, 64, [[NR, 9], [1, 32]]))
            A.copy(_ap(US[:], 0, [[128, 9], [1, 32]]),
                   _ap(vmv, 0, [[NR, 9], [1, 32]]))

            def W2L(k, c0, c1_):
                return _ap(P2e2[:], k * 64 + c0,
                           [[192, 3], [0, 3], [1, c1_ - c0]])

            def W2R(k, c0, c1_):
                return _ap(WS[:], k * 64 + c0,
                           [[0, 3], [192, 3], [1, c1_ - c0]])

            def W2O(c0, c1_):
                return _ap(US[:], 64 + c0,
                           [[128, 3], [384, 3], [1, c1_ - c0]])

            def W2T(t, c0, c1_):
                return _ap(t[:], 288 + c0,
                           [[64, 3], [192, 3], [1, c1_ - c0]])

            fused(W2L, W2R, W2O, W2T, 2 * NB, split=0.8)

        # --- Hillis-Steele over 32 quads, expansions interleaved
        def hs_step(srcb, dstb, s):
            n = NB - s
            sv = srcb.rearrange("p (e j) -> p e j", e=9)
            dv = dstb.rearrange("p (e j) -> p e j", e=9)
            V.tensor_copy(dv[:, :, 0:s], sv[:, :, 0:s])

            def HL(k, c0, c1_):
                return _ap(srcb, k * NB + c0,
                           [[96, 3], [0, 3], [1, c1_ - c0]])

            def HR(k, c0, c1_):
                return _ap(srcb, 3 * k * NB + s + c0,
                           [[0, 3], [32, 3], [1, c1_ - c0]])

            def HO(c0, c1_):
                return _ap(dstb, s + c0, [[96, 3], [32, 3], [1, c1_ - c0]])

            def HT(t, c0, c1_):
                return _ap(t[:], c0, [[96, 3], [32, 3], [1, c1_ - c0]])

            fused(HL, HR, HO, HT, n, split=0.8)

        def emit_u(u0, u1):
            def UL(k, c0, c1_):
                return _ap(L4[:], k * 128 + u0 + c0,
                           [[384, 3], [0, 3], [1, c1_ - c0]])

            def UR(k, c0, c1_):
                return _ap(US[:], k * 128 + u0 + c0,
                           [[0, 3], [384, 3], [1, c1_ - c0]])

            def UO(c0, c1_):
                return _ap(Uloc[:], u0 + c0,
                           [[384, 3], [128, 3], [1, c1_ - c0]])

            def UT(t, c0, c1_):
                return _ap(t[:], u0 + c0,
                           [[384, 3], [128, 3], [1, c1_ - c0]])

            fused(UL, UR, UO, UT, u1 - u0, split=0.8)

        bufs = [P4A, P4B]
        emit_wodd(0, G)
        hs_step(bufs[0][:], bufs[1][:], 1)
        hs_step(bufs[1][:], bufs[0][:], 2)
        hs_step(bufs[0][:], bufs[1][:], 4)
        hs_step(bufs[1][:], bufs[0][:], 8)
        hs_step(bufs[0][:], bufs[1][:], 16)
        emit_wodd(1, G)
        Rscan = bufs[1][:]    # P4B
        # L4 needs only Rscan: emit before W2 to keep its wait early
        A.copy(_ap(L4[:], 1, [[128, 9], [32, 4], [1, NB - 1]]),
               _ap(Rscan, 0, [[NB, 9], [0, 4], [1, NB - 1]]))
        emit_u(0, 2 * NB)       # g0/g1 half (needs only Wodd b0 + vm)
        emit_w2()

        # --- rotation fixup: E gathered to ALL rows, F chain, slice Fbc ---
        for q in range(QN):
            nc.tensor.matmul(
                PSg[:, q * 9:(q + 1) * 9],
                selbq[:, q * 128:(q + 1) * 128],
                _ap(Rscan, NB - 1, [[NB, 9]]), start=True, stop=True)
        V.tensor_copy(Estack[:, 0:36], PSg[:, 0:36])
        V.tensor_copy(Fstack[:, 0:9], Estack[:, 0:9])
        fs = Fstack[:]
        es = Estack[:]

        def ap2(base_ap, off, dims):
            return _ap(base_ap, off, dims)

        for q in (1, 2):
            FL = lambda k: ap2(fs, (q - 1) * 9 + k, [[3, 3], [0, 3]])
            ER = lambda k: ap2(es, q * 9 + 3 * k, [[0, 3], [1, 3]])
            MT = lambda t: ap2(t[:], 0, [[3, 3], [1, 3]])
            FO = ap2(fs, q * 9, [[3, 3], [1, 3]])
            V.tensor_mul(MT(mt0), FL(0), ER(0))
            V.tensor_mul(MT(mt1), FL(1), ER(1))
            V.tensor_add(MT(mt0), MT(mt0), MT(mt1))
            V.tensor_mul(MT(mt1), FL(2), ER(2))
            V.tensor_add(FO, MT(mt0), MT(mt1))
        # Fbc: rows 0:32 identity; quarter q rows take F_q slice
        V.memset(Fbc[0:BPC, 0:9], 0.0)
        V.memset(bass.AP(Fbc[:].tensor, Fbc[:].offset,
                         [[Fbc[:].ap[0][0], BPC], [4, 3]]), 1.0)
        for q in (1, 2, 3):
            V.tensor_copy(Fbc[q * BPC:(q + 1) * BPC, 0:9],
                          Fstack[q * BPC:(q + 1) * BPC,
                                 (q - 1) * 9:q * 9])

        # --- U merged: Uloc[c][j*128 + g*32 + r] = L4 @ US
        emit_u(2 * NB, 4 * NB)  # g2/g3 half (needs W2)

        # seeds: r=0 of each g-block: identity prefix -> u = US col g*32
        A.copy(_ap(Uloc[:], 0, [[384, 3], [128, 3], [32, 4]]),
               _ap(US[:], 0, [[128, 3], [384, 3], [32, 4]]))
        # atom 0 of whole chain (q=0 rows) at origin
        V.memset(bass.AP(Uloc[:].tensor, Uloc[:].offset,
                         [[Uloc[:].ap[0][0], BPC], [384, 3]]), 0.0)

        # ============ Phase G: tail ============
        # F-apply fused with blocked->ordered reorder:
        #   Uord_c[a] = F_c0*Ux[b(a)] + F_c1*Uy[b(a)] + F_c2*Uz[b(a)]
        # (walk order (r,g,j): out stride-1-ish ordered, ins blocked)
        def ordw(t, c):
            return _ap(t[:], c * W, [[12, 32], [3, 4], [1, 3]])

        def blkw(t, c):
            return _ap(t[:], c * W, [[1, 32], [32, 4], [128, 3]])

        for c in range(3):
            x = T9a[:, c * W:(c + 1) * W]    # fp16 scratch (blocked)
            y = T9b[:, c * W:(c + 1) * W]
            z = T9c[:, c * W:(c + 1) * W]
            A.mul(x, Uloc[:, 0:W], Fbc[:, 3 * c + 0:3 * c + 1])
            TS(y, Uloc[:, W:2 * W], Fbc[:, 3 * c + 1:3 * c + 2], None,
               op0=OP.mult)
            GTS(z, Uloc[:, 2 * W:3 * W], Fbc[:, 3 * c + 2:3 * c + 3], None,
                op0=OP.mult)
            V.tensor_add(x, x, y)
            RS = 20
            V.tensor_add(_ap(Uord[:], c * W, [[12, RS], [3, 4], [1, 3]]),
                         _ap(x, 0, [[1, RS], [32, 4], [128, 3]]),
                         _ap(z, 0, [[1, RS], [32, 4], [128, 3]]))
            G.tensor_add(
                _ap(Uord[:], c * W + 12 * RS, [[12, 32 - RS], [3, 4], [1, 3]]),
                _ap(x, RS, [[1, 32 - RS], [32, 4], [128, 3]]),
                _ap(z, RS, [[1, 32 - RS], [32, 4], [128, 3]]))

        # quarter sums of rotated u -> Pinc via gather + cumsum
        for c in range(3):
            V.tensor_reduce(Sall[:, c:c + 1], Uord[:, c * W:(c + 1) * W],
                            axis=mybir.AxisListType.X, op=OP.add)
        V.tensor_copy(Sall16[:], Sall[:])
        for q in range(QN):
            nc.tensor.matmul(
                PSp[:, q * 3:(q + 1) * 3],
                selbq[:, q * 128:(q + 1) * 128],
                Sall16[:, 0:3], start=True, stop=True)
        V.tensor_copy(Sg[:, 0:9], PSp[:, 0:9])
        # cumulative sums: cumst block q-1 = Pinc_{q}
        V.tensor_copy(cumst[:, 0:3], Sg[:, 0:3])
        V.tensor_add(cumst[:, 3:6], cumst[:, 0:3], Sg[:, 3:6])
        V.tensor_add(cumst[:, 6:9], cumst[:, 3:6], Sg[:, 6:9])
        # Pincb: rows 0:32 zero; quarter q rows take Pinc_q slice
        V.memset(FtPb[0:BPC, 0:3], 0.0)
        for q in (1, 2, 3):
            V.tensor_copy(FtPb[q * BPC:(q + 1) * BPC, 0:3],
                          cumst[q * BPC:(q + 1) * BPC, (q - 1) * 3:q * 3])

        # masked scan seeded with Pinc -> final coords; store
        for c in range(3):
            V.tensor_tensor_scan(
                Pall[:, c * W:(c + 1) * W], Uord[:, c * W:(c + 1) * W],
                maskp[:], FtPb[:, c:c + 1], op0=OP.add, op1=OP.mult)
            (nc.sync if c != 1 else nc.scalar).dma_start(
                out[:, c * W:(c + 1) * W], Pall[:, c * W:(c + 1) * W])

def _prep_alpha(input):
    # alphaN[r]=psi[r-1], alphaCA[r]=omega[r-1] (0 at r=0), alphaC[r]=phi[r];
    # then mod-4 block permutation within each 128-residue quarter.
    phi, psi, om = input[:, 0], input[:, 1], input[:, 2]
    z1 = np.zeros((input.shape[0], 1), np.float32)
    aN = np.concatenate([z1, psi[:, :-1]], axis=1)
    aCA = np.concatenate([z1, om[:, :-1]], axis=1)
    alpha = np.stack([aN, aCA, phi], axis=1)          # [B, 3, 512]
    alpha = alpha.reshape(-1, 3, QN, NR)
    perm = np.arange(NR).reshape(NB, 4).T.reshape(-1)  # sigma^-1: col->r
    alpha = alpha[..., perm]                           # blocked columns
    return alpha.transpose(0, 2, 1, 3)                 # [B, QN, 3, NR]


def _shard_alpha(alpha, i):
    sl = slice(i * BPC, (i + 1) * BPC)
    return np.ascontiguousarray(
        alpha[sl].transpose(1, 0, 2, 3).reshape(QN * BPC, 3 * NR))


def _get_nc():
    if "nc" not in _CACHE:
        _CACHE["nc"] = _build_graph()
    return _CACHE["nc"]


def kernel(input, param, angles_length, trace=False):
    input = np.ascontiguousarray(input, dtype=np.float32)
    param = np.ascontiguousarray(param, dtype=np.float32)
    angles_length = np.ascontiguousarray(angles_length, dtype=np.int32)
    nc = _get_nc()
    alpha = _prep_alpha(input)
    in_maps = []
    for i in range(NCORES):
        sl = slice(i * BPC, (i + 1) * BPC)
        in_maps.append({
            "input": _shard_alpha(alpha, i),
            "param": param,
            "angles_length": angles_length[sl],
        })
    res = run_bass_kernel_spmd(nc, in_maps, core_ids=list(range(NCORES)),
                               trace=trace)
    outs = []
    for i in range(NCORES):
        r = res.results[i]["out"]          # [(q,b), (c,j)]
        r = r.reshape(QN, BPC, 3, W)
        r = np.transpose(r, (1, 0, 3, 2)).reshape(BPC, 3 * QN * W)
        outs.append(r)
    full = np.concatenate(outs, axis=0).astype(np.float32)
    if trace:
        kernel._last_exec_ns = res.exec_time_ns
    return full


kernel._last_exec_ns = None
